# revision 43
# baseline (speedup 1.0000x reference)
"""BGE-M3 scoring kernel for 8 Trainium2 NeuronCores.

Data-parallel over the 64 passages (8 per core); query side replicated.
Each core produces the [8, 8] column block of dense/sparse/colbert scores
for its passages; the host concatenates blocks along axis 1.

Self-contained: builds the Bass program once (module cache) and runs it
via run_bass_kernel_spmd on cores 0-7.
"""
import numpy as np
import concourse.bass as bass
import concourse.tile as tile
import concourse.mybir as mybir
from concourse.bass_utils import run_bass_kernel_spmd
from concourse.vector_clock import ScopedClock

F32 = mybir.dt.float32
F32R = mybir.dt.float32r
BF16 = mybir.dt.bfloat16
F8 = mybir.dt.float8e4
PM = mybir.MatmulPerfMode
AX = mybir.AluOpType
AF = mybir.ActivationFunctionType
X = mybir.AxisListType.X
FP8_SCALE = 16.0  # colbert unit vectors are scaled by this before fp8 cast
W_SCALE = 64.0    # colbert_w is scaled by this before fp8 cast (sigma 0.02)

N_CORES = 8
H = 1024
BQ, LQ = 8, 128
BP_FULL, LP = 64, 512
BP = BP_FULL // N_CORES          # 8 passages per core
HC = H // 128                    # 8 chunks of the hidden dim
TEMP = 0.02

# ---------------------------------------------------------------------------
# Walrus workaround: this container's neuronxcc rejects >1 sem wait per
# instruction ("Too many sync wait commands"). Split extra waits onto
# single-wait NOPs inserted just before the instruction on the same engine.
# ---------------------------------------------------------------------------
_wait_counter = [0]


def _split_multi_waits(nc):
    for fn in nc.m.functions:
        for bb in fn.blocks:
            out, changed = [], False
            for inst in bb.instructions:
                si = inst.sync_info
                if si is not None and len(si.on_wait) > 1:
                    changed = True
                    waits = list(si.on_wait)
                    for w in waits[:-1]:
                        _wait_counter[0] += 1
                        nop = mybir.InstNoOp(
                            name=f"I-waitsplit-{_wait_counter[0]}", ins=[], outs=[])
                        nop.engine = inst.engine
                        nop.sync_info = mybir.SyncInfo(on_wait=[w], on_update=[])
                        nc.register_instruction(nop)
                        out.append(nop)
                    inst.sync_info = mybir.SyncInfo(
                        on_wait=[waits[-1]], on_update=list(si.on_update))
                out.append(inst)
            if changed:
                bb.instructions = out


class _TC(tile.TileContext):
    def _drain_and_barrier(self, tick_clock, wait_clock):
        nc = self.nc
        drain_inst = nc.sync.drain()
        wait_clock.add_sem_waits(
            drain_inst.ins, ScopedClock({None: tick_clock.global_clock}))
        nc.all_engine_barrier()
        assert self.sems is not None
        popped = nc._tile_sem_poison_stack.pop()
        assert popped is self._sem_poison
        nc.clear_and_free_semaphores(list(self.sems.allocated().values()))
        nc.all_engine_barrier()

    def __exit__(self, *args):
        r = super().__exit__(*args)
        _split_multi_waits(self.nc)
        return r


def _bcast_rows(row_ap, parts=128):
    """DMA source AP replicating one DRAM row across `parts` partitions."""
    return bass.AP(tensor=row_ap.tensor, offset=row_ap.offset,
                   ap=[[0, parts]] + [list(d) for d in row_ap.ap])


# ---------------------------------------------------------------------------
# Program construction
# ---------------------------------------------------------------------------
def _build_program(repeats=1):
    nc = bass.Bass()

    # DRAM I/O (per core). Hidden states arrive host-pre-transposed
    # (chunk-major, hidden-on-partitions): bf16 full-precision copies for the
    # sparse/dense paths, fp8 pair layouts (DoubleRow) for the projections.
    d_qhT = nc.dram_tensor("q_hiddenT", [128, HC, BQ * LQ], BF16,
                           kind="ExternalInput")
    d_phT = nc.dram_tensor("p_hiddenT", [BP, 128, HC, LP], BF16,
                           kind="ExternalInput")
    d_q8 = nc.dram_tensor("q_hidden8", [128, HC // 2, 2, BQ * LQ], F8,
                          kind="ExternalInput")
    d_p8 = nc.dram_tensor("p_hidden8", [BP, 128, HC // 2, 2, LP], F8,
                          kind="ExternalInput")
    d_w8 = nc.dram_tensor("colbert_w8", [128, HC // 2, 2, H], F8,
                          kind="ExternalInput")
    d_qm = nc.dram_tensor("q_mask", [BQ, LQ], F32R, kind="ExternalInput")
    d_pm = nc.dram_tensor("p_mask", [BP, LP], F32R, kind="ExternalInput")
    d_qi = nc.dram_tensor("q_ids_f", [BQ, LQ], F32, kind="ExternalInput")
    d_pi = nc.dram_tensor("p_ids_f", [BP, LP], F32, kind="ExternalInput")
    d_cb = nc.dram_tensor("colbert_b", [H], F32, kind="ExternalInput")
    d_sw = nc.dram_tensor("sparse_w", [H], BF16, kind="ExternalInput")
    d_sb = nc.dram_tensor("sparse_b", [1, 1], F32, kind="ExternalInput")
    d_oc = nc.dram_tensor("ones_col", [128, 1], F32R, kind="ExternalInput")
    d_op = nc.dram_tensor("ones_pair", [128, 2, 1], F8, kind="ExternalInput")
    d_or = nc.dram_tensor("ones_row", [1, 128], F32R, kind="ExternalInput")
    d_lm = nc.dram_tensor("lmask", [128, 128], F32, kind="ExternalInput")

    o_dense = nc.dram_tensor("dense", [BQ, BP], F32, kind="ExternalOutput")
    o_sparse = nc.dram_tensor("sparse", [BQ, BP], F32, kind="ExternalOutput")
    o_colbert = nc.dram_tensor("colbert", [BQ, BP], F32, kind="ExternalOutput")

    with _TC(nc) as tc:
        for _ in range(repeats):
            _emit(nc, tc, d_qhT, d_phT, d_q8, d_p8, d_w8, d_qm, d_pm, d_qi,
                  d_pi, d_cb, d_sw, d_sb, d_oc, d_op, d_or, d_lm, o_dense,
                  o_sparse, o_colbert)
    return nc


def _emit(nc, tc, d_qhT, d_phT, d_q8, d_p8, d_w8, d_qm, d_pm, d_qi, d_pi,
          d_cb, d_sw, d_sb, d_oc, d_op, d_or, d_lm, o_dense, o_sparse,
          o_colbert):
    from contextlib import ExitStack
    es = ExitStack()
    with es:
        es.enter_context(nc.allow_low_precision(reason="fp22/f32r is the target precision"))
        # ---- pools -------------------------------------------------------
        persist = es.enter_context(tc.tile_pool(name="persist", bufs=1))
        wt_pool = es.enter_context(tc.tile_pool(name="wt", bufs=1))
        qcolT_pool = es.enter_context(tc.tile_pool(name="qcolT", bufs=1))
        dram = es.enter_context(tc.tile_pool(name="dram", bufs=1, space="DRAM"))
        ps_mm = es.enter_context(tc.tile_pool(name="ps_mm", bufs=3, space="PSUM"))
        ps_ss = es.enter_context(tc.tile_pool(name="ps_ss", bufs=2, space="PSUM"))
        ps_misc = es.enter_context(tc.tile_pool(name="ps_misc", bufs=1, space="PSUM"))

        # ---- persistent small tiles --------------------------------------
        ones_c = persist.tile([128, 1], F32R, tag="ones_c")
        nc.sync.dma_start(out=ones_c[:], in_=d_oc[:])
        ones_p = persist.tile([128, 2, 1], F8, tag="ones_p")
        nc.sync.dma_start(out=ones_p[:], in_=d_op[:])
        ones_r = persist.tile([1, 128], F32R, tag="ones_r")
        nc.sync.dma_start(out=ones_r[:], in_=d_or[:])
        lmask = persist.tile([128, 128], F32, tag="lmask")
        nc.sync.dma_start(out=lmask[:], in_=d_lm[:])
        sb_sb = persist.tile([1, 1], F32, tag="sb")
        nc.sync.dma_start(out=sb_sb[:], in_=d_sb[:])
        # colbert bias chunks: cb_sb[:, m] = b[m*128:(m+1)*128]
        cb_sb = persist.tile([128, HC], F32, tag="cb")
        nc.sync.dma_start(out=cb_sb[:], in_=d_cb.ap().rearrange("(m p) -> p m", p=128))
        # sparse_w chunks as lhsT columns: sw_sb[:, k] = sw[k*128:(k+1)*128]
        sw_sb = persist.tile([128, HC], BF16, tag="sw")
        nc.sync.dma_start(out=sw_sb[:], in_=d_sw.ap().rearrange("(k p) -> p k", p=128))
        # q ids as per-token columns: qid_cols[:, i] = q_ids_f[i, :]
        qid_cols = persist.tile([128, BQ], F32, tag="qid_cols")
        nc.sync.dma_start(out=qid_cols[:], in_=d_qi.ap().rearrange("i l -> l i"))
        # q mask tokens 1..127 transposed [token-1, batch] for qlen
        qmT = persist.tile([128, BQ], F32R, tag="qmT")
        nc.sync.dma_start(
            out=qmT[0:127, :],
            in_=bass.AP(tensor=d_qm.ap().tensor, offset=1,
                        ap=[[1, 127], [128, BQ]]))

        qcls_t = persist.tile([128, HC, BQ], BF16, tag="qcls")
        qcls = [qcls_t[:, k, :] for k in range(HC)]
        pcls_t = persist.tile([128, HC, BP], BF16, tag="pcls")
        pcls = [pcls_t[:, k, :] for k in range(HC)]
        rmax = [persist.tile([128, BP], F32R, tag=f"rmax{i}", name=f"rmax{i}")
                for i in range(BQ)]
        smax = [persist.tile([128, BP], F32R, tag=f"smax{i}", name=f"smax{i}")
                for i in range(BQ)]
        qw = [persist.tile([128, 1], F32R, tag=f"qw{i}", name=f"qw{i}")
              for i in range(BQ)]
        # per-(token,query) colbert final scale columns
        qnr = persist.tile([128, BQ], F32R, tag="qnr")
        twq_cols = persist.tile([128, BQ], F32R, tag="twq_cols")

        # fp8 pair-layout W (x W_SCALE): w8[g][:, t, m] = W[m, (2g+t)*128+p]
        w8_t = wt_pool.tile([128, HC // 2, 2, H], F8, tag="w8")
        w8 = [w8_t[:, g, :, :] for g in range(HC // 2)]
        # fp8 pair-layout q colbert vectors: qcolT8[g][:, t, :] holds hidden
        # chunk m=2g+t, scaled by FP8_SCALE (DoubleRow contracts 2x128=256)
        qcolT8 = [qcolT_pool.tile([128, 2, BQ * LQ], F8, tag=f"qct{g}",
                                  name=f"qct{g}") for g in range(HC // 2)]

        d_twq = dram.tile([1, BQ * LQ], F32R, name="d_twq")
        d_qnr = dram.tile([1, BQ * LQ], F32R, name="d_qnr")
        d_rq = dram.tile([1, BQ], F32, name="d_rq")

        # ================= SETUP: W8 (host-pre-transposed fp8 pairs) ======
        nc.sync.dma_start(out=w8_t[:], in_=d_w8.ap())

        # ================= SETUP: q side ==================================
        with tc.tile_pool(name="qhT", bufs=1) as qhT_pool, \
             tc.tile_pool(name="qtmp", bufs=1) as qtmp_pool, \
             tc.tile_pool(name="qv", bufs=2) as qv_pool:
            qhT_t = qhT_pool.tile([128, HC, BQ * 128], BF16, tag="qhT")
            nc.sync.dma_start(out=qhT_t[:], in_=d_qhT.ap())
            qhT = [qhT_t[:, k, :] for k in range(HC)]
            q8_t = qhT_pool.tile([128, HC // 2, 2, BQ * LQ], F8, tag="q8")
            nc.scalar.dma_start(out=q8_t[:], in_=d_q8.ap())
            q8 = [q8_t[:, g, :, :] for g in range(HC // 2)]
            # CLS columns (token 0 of each batch), all chunks in one copy
            nc.scalar.copy(
                out=qcls_t[:],
                in_=qhT_t.rearrange("p k (i l) -> p k i l", i=BQ)[:, :, :, 0])

            # token weights tw_q = relu(qh . sw + b), all 128 tokens per batch
            twq_row = qtmp_pool.tile([1, BQ * 128], F32R, tag="twq")
            for g in range(2):
                ptw = ps_ss.tile([1, 512], F32, tag="ss")
                for k in range(HC):
                    nc.tensor.matmul(ptw[:], sw_sb[:, k:k + 1],
                                     qhT[k][:, g * 512:(g + 1) * 512],
                                     start=(k == 0), stop=(k == HC - 1))
                nc.scalar.activation(out=twq_row[:, g * 512:(g + 1) * 512],
                                     in_=ptw[:], func=AF.Relu, bias=sb_sb[:], scale=1.0)
            # column form via DRAM bounce: twq_cols[l, i] = tw_q[i, l]
            nc.sync.dma_start(out=d_twq[:], in_=twq_row[:])
            nc.sync.dma_start(
                out=twq_cols[:],
                in_=bass.AP(tensor=d_twq.tensor, offset=0, ap=[[1, 128], [128, BQ]]))

            # ---- q dedup: keep, per (batch, id), the max-tw token --------
            for i in range(BQ):
                pb = ps_misc.tile([128, 128], F32, tag="misc")
                nc.tensor.matmul(pb[:], ones_r[:],
                                 twq_row[:, i * 128:(i + 1) * 128],
                                 start=True, stop=True)
                twB = qtmp_pool.tile([128, 128], F32, tag="twB")
                nc.scalar.copy(out=twB[:], in_=pb[:])
                tw_col = twq_cols[:, i:i + 1]
                qidB = qtmp_pool.tile([128, 128], F32, tag="qidB")
                nc.gpsimd.dma_start(out=qidB[:],
                                    in_=_bcast_rows(d_qi[i:i + 1, :]))
                # dominated(a) = sum_a' eq_id*(gt_tw + eq_tw*lower)
                x2 = qtmp_pool.tile([128, 128], F32, tag="x2")
                nc.vector.scalar_tensor_tensor(
                    out=x2[:], in0=twB[:], scalar=tw_col, in1=lmask[:],
                    op0=AX.is_equal, op1=AX.mult)
                x3 = qtmp_pool.tile([128, 128], F32, tag="x3")
                nc.vector.scalar_tensor_tensor(
                    out=x3[:], in0=twB[:], scalar=tw_col, in1=x2[:],
                    op0=AX.is_gt, op1=AX.add)
                dom = qtmp_pool.tile([128, 128], F32, tag="dom")
                dsum = qtmp_pool.tile([128, 1], F32, tag="dsum")
                nc.vector.scalar_tensor_tensor(
                    out=dom[:], in0=qidB[:], scalar=qid_cols[:, i:i + 1], in1=x3[:],
                    op0=AX.is_equal, op1=AX.mult, accum_out=dsum[:])
                surv = qtmp_pool.tile([128, 1], F32, tag="surv")
                nc.vector.tensor_scalar(out=surv[:], in0=dsum[:], scalar1=0.0,
                                        scalar2=None, op0=AX.is_equal)
                nc.vector.tensor_mul(qw[i][:], surv[:], tw_col)

            # ---- q colbert projection: raw v cast straight to fp8 pairs;
            # per-token 1/norm goes into the qnr finals column instead.
            qn_row = qtmp_pool.tile([1, BQ * 128], F32, tag="qn_row")
            for g in range(2):
                vsq8 = [qv_pool.tile([128, 2, 512], F8, tag=f"qvs{gg}",
                                     name=f"qvs{gg}") for gg in range(HC // 2)]
                pss = ps_ss.tile([1, 512], F32, tag="ss")
                for m in range(HC):
                    pmm = ps_mm.tile([128, 512], F32, tag="mm")
                    for kg in range(HC // 2):
                        nc.tensor.matmul(pmm[:],
                                         w8[kg][:, :, m * 128:(m + 1) * 128],
                                         q8[kg][:, :, g * 512:(g + 1) * 512],
                                         start=(kg == 0), stop=(kg == HC // 2 - 1),
                                         perf_mode=PM.DoubleRow)
                    nc.scalar.activation(
                        out=qcolT8[m // 2][:, m % 2, g * 512:(g + 1) * 512],
                        in_=pmm[:], func=AF.Identity,
                        bias=cb_sb[:, m:m + 1], scale=1.0 / W_SCALE)
                    nc.scalar.activation(out=vsq8[m // 2][:, m % 2, :], in_=pmm[:],
                                         func=AF.Square,
                                         bias=cb_sb[:, m:m + 1], scale=1.0 / W_SCALE)
                for gg in range(HC // 2):
                    for t in range(2):
                        nc.tensor.matmul(pss[:], ones_p[:, 0, :],
                                         vsq8[gg][:, t, :],
                                         start=(gg == 0 and t == 0),
                                         stop=(gg == HC // 2 - 1 and t == 1))
                nc.scalar.activation(out=qn_row[:, g * 512:(g + 1) * 512],
                                     in_=pss[:], func=AF.Sqrt)
            # qnr0 = qmask / ||v|| as a [1, BQ*LQ] row
            rq_full = qtmp_pool.tile([1, BQ * 128], F32R, tag="rq_full")
            nc.vector.reciprocal(out=rq_full[:], in_=qn_row[:])
            mrow = qtmp_pool.tile([1, BQ * 128], F32R, tag="mrow")
            nc.sync.dma_start(
                out=mrow[:],
                in_=bass.AP(tensor=d_qm.ap().tensor, offset=0,
                            ap=[[0, 1], [1, BQ * 128]]))
            nc.vector.tensor_mul(rq_full[:], rq_full[:], mrow[:])

            # qnr[a-1, i] = qmask[i,a] / (||v_q[i,a]|| * qlen_i * TEMP)
            # (tokens 1..127 on partitions 0..126, via DRAM bounce)
            nc.sync.dma_start(out=d_qnr[:], in_=rq_full[:])
            nc.sync.dma_start(
                out=qnr[0:127, :],
                in_=bass.AP(tensor=d_qnr.tensor, offset=1,
                            ap=[[1, 127], [128, BQ]]))
            pql = ps_ss.tile([1, BQ], F32, tag="ss")
            nc.tensor.matmul(pql[:], ones_c[0:127, :], qmT[0:127, :],
                             start=True, stop=True)
            qiv_row = qtmp_pool.tile([1, BQ], F32R, tag="qiv")
            nc.vector.tensor_scalar(out=qiv_row[:], in0=pql[:],
                                    scalar1=TEMP * FP8_SCALE,
                                    scalar2=None, op0=AX.mult)
            nc.vector.reciprocal(out=qiv_row[:], in_=qiv_row[:])
            pqb = ps_misc.tile([127, BQ], F32, tag="misc")
            nc.tensor.matmul(pqb[:], ones_r[:, 0:127], qiv_row[:],
                             start=True, stop=True)
            qivB = qtmp_pool.tile([128, BQ], F32, tag="qivB")
            nc.scalar.copy(out=qivB[0:127, :], in_=pqb[:])
            nc.vector.tensor_mul(qnr[0:127, :], qnr[0:127, :], qivB[0:127, :])

        # ================= MAIN LOOP over passages ========================
        # Software-pipelined: stage A(j) = DMA+transpose+project+normalize
        # (produces pcolT_j, twpB_j, pidB_j), stage B(j) = colbert scores +
        # sparse match. Emission order A0, A1, B0, A2, B1, ... keeps the PE
        # stream free of stalls: B(j)'s operands are ready by the time the
        # in-order PE queue reaches them.
        with tc.tile_pool(name="phT", bufs=2) as phT_pool, \
             tc.tile_pool(name="pcs", bufs=2) as pcs_pool, \
             tc.tile_pool(name="pvq", bufs=2) as pvq_pool, \
             tc.tile_pool(name="pcolT", bufs=2) as pcolT_pool, \
             tc.tile_pool(name="prow2", bufs=2) as prow2_pool, \
             tc.tile_pool(name="prow", bufs=1) as prow_pool:

            def stage_a(j):
                phT_t = phT_pool.tile([128, HC, LP], BF16, tag="phT")
                nc.sync.dma_start(out=phT_t[:], in_=d_phT[j])
                phT = [phT_t[:, k, :] for k in range(HC)]
                p8 = phT_pool.tile([128, HC // 2, 2, LP], F8, tag="p8")
                nc.sync.dma_start(out=p8[:], in_=d_p8[j])
                nc.scalar.copy(out=pcls_t[:, :, j], in_=phT_t[:, :, 0])

                # token weights tw_p = relu(ph . sw + b)
                ptw = ps_ss.tile([1, LP], F32, tag="ss")
                for k in range(HC):
                    nc.tensor.matmul(ptw[:], sw_sb[:, k:k + 1], phT[k][:],
                                     start=(k == 0), stop=(k == HC - 1))
                twp_row = prow_pool.tile([1, LP], F32R, tag="twp")
                nc.scalar.activation(out=twp_row[:], in_=ptw[:], func=AF.Relu,
                                     bias=sb_sb[:], scale=1.0)
                ptb = ps_misc.tile([128, LP], F32, tag="misc")
                nc.tensor.matmul(ptb[:], ones_r[:], twp_row[:], start=True, stop=True)
                twpB = prow2_pool.tile([128, LP], F32, tag="twpB")
                nc.scalar.copy(out=twpB[:], in_=ptb[:])
                pidB = prow2_pool.tile([128, LP], F32, tag="pidB")
                nc.gpsimd.dma_start(out=pidB[:], in_=_bcast_rows(d_pi[j:j + 1, :]))

                # colbert projection (all 512 tokens): raw v -> fp8 pairs
                pcolT8 = [pcolT_pool.tile([128, 2, LP], F8, tag=f"pct{g}",
                                          name=f"pct{g}_{j}")
                          for g in range(HC // 2)]
                vsq8 = [pvq_pool.tile([128, 2, LP], F8, tag=f"pvs{gg}",
                                      name=f"pvs{gg}") for gg in range(HC // 2)]
                pss = ps_ss.tile([1, LP], F32, tag="ss")
                for m in range(HC):
                    pmm = ps_mm.tile([128, LP], F32, tag="mm")
                    for kg in range(HC // 2):
                        nc.tensor.matmul(pmm[:],
                                         w8[kg][:, :, m * 128:(m + 1) * 128],
                                         p8[:, kg, :, :],
                                         start=(kg == 0), stop=(kg == HC // 2 - 1),
                                         perf_mode=PM.DoubleRow)
                    nc.scalar.activation(out=pcolT8[m // 2][:, m % 2, :],
                                         in_=pmm[:], func=AF.Identity,
                                         bias=cb_sb[:, m:m + 1], scale=1.0 / W_SCALE)
                    nc.scalar.activation(out=vsq8[m // 2][:, m % 2, :], in_=pmm[:],
                                         func=AF.Square,
                                         bias=cb_sb[:, m:m + 1], scale=1.0 / W_SCALE)
                for gg in range(HC // 2):
                    for t in range(2):
                        nc.tensor.matmul(pss[:], ones_p[:, 0, :],
                                         vsq8[gg][:, t, :],
                                         start=(gg == 0 and t == 0),
                                         stop=(gg == HC // 2 - 1 and t == 1))
                nrow = prow_pool.tile([1, LP], F32, tag="nrow")
                nc.scalar.activation(out=nrow[:], in_=pss[:], func=AF.Sqrt,
                                     scale=1.0 / (FP8_SCALE * FP8_SCALE))
                rrow = prow_pool.tile([1, LP], F32, tag="rrow")
                nc.vector.reciprocal(out=rrow[:], in_=nrow[:])
                rp_row = prow_pool.tile([1, LP], F32R, tag="rp_row")
                mrow = prow_pool.tile([1, LP], F32R, tag="mrow")
                nc.scalar.dma_start(out=mrow[:], in_=d_pm[j:j + 1, :])
                nc.vector.tensor_mul(rp_row[:], rrow[:], mrow[:])
                pbc = ps_misc.tile([128, LP], F32, tag="misc")
                nc.tensor.matmul(pbc[:], ones_r[:], rp_row[:], start=True, stop=True)
                rpB = prow2_pool.tile([128, LP], F32, tag="rpB")
                nc.scalar.copy(out=rpB[:], in_=pbc[:])
                # normalize on Pool: pcolT8s = fp8(v8 * FP8_SCALE*mask/||v||)
                pcolT8s = [pcs_pool.tile([128, 2, LP], F8, tag=f"pcs{g}",
                                         name=f"pcs{g}_{j}")
                           for g in range(HC // 2)]
                for g in range(HC // 2):
                    for t in range(2):
                        nc.gpsimd.tensor_mul(pcolT8s[g][:, t, :],
                                             pcolT8[g][:, t, :], rpB[:])
                return pcolT8s, twpB, pidB

            def stage_b(j, st):
                pcolT8s, twpB, pidB = st
                for i in range(BQ):
                    psc = ps_mm.tile([127, LP], F32, tag="mm")
                    for g in range(HC // 2):
                        nc.tensor.matmul(
                            psc[:],
                            qcolT8[g][:, :, i * 128 + 1:(i + 1) * 128],
                            pcolT8s[g][:, :, :],
                            start=(g == 0), stop=(g == HC // 2 - 1),
                            perf_mode=PM.DoubleRow)
                    nc.vector.reduce_max(out=rmax[i][0:127, j:j + 1],
                                         in_=psc[:, 1:LP], axis=X)
                    eqw = prow_pool.tile([128, LP], F32, tag="eqw")
                    nc.gpsimd.tensor_scalar(
                        out=eqw[:], in0=pidB[:], scalar1=qid_cols[:, i:i + 1],
                        scalar2=None, op0=AX.is_equal)
                    mt = prow_pool.tile([128, LP], F32, tag="mt")
                    nc.gpsimd.tensor_mul(mt[:], eqw[:], twpB[:])
                    nc.vector.reduce_max(out=smax[i][:, j:j + 1], in_=mt[:], axis=X)

            pending = stage_a(0)
            for j in range(1, BP):
                nxt = stage_a(j)
                stage_b(j - 1, pending)
                pending = nxt
            stage_b(BP - 1, pending)

        # ================= FINALS =========================================
        with tc.tile_pool(name="fin", bufs=1) as fin:
            for i in range(BQ):
                pcbi = ps_ss.tile([1, BP], F32, tag="ss")
                nc.tensor.matmul(pcbi[:], qnr[0:127, i:i + 1],
                                 rmax[i][0:127, :], start=True, stop=True)
                stag = fin.tile([1, BP], F32, tag=f"cst{i}", name=f"cst{i}")
                nc.scalar.copy(out=stag[:], in_=pcbi[:])
                nc.sync.dma_start(out=o_colbert[i:i + 1, :], in_=stag[:])

                pspi = ps_ss.tile([1, BP], F32, tag="ss")
                nc.tensor.matmul(pspi[:], qw[i][:], smax[i][:],
                                 start=True, stop=True)
                stag2 = fin.tile([1, BP], F32, tag=f"sst{i}", name=f"sst{i}")
                nc.scalar.activation(out=stag2[:], in_=pspi[:], func=AF.Copy,
                                     scale=1.0 / TEMP)
                nc.sync.dma_start(out=o_sparse[i:i + 1, :], in_=stag2[:])

            # dense scores
            pd = ps_misc.tile([BQ, BP], F32, tag="misc")
            pqn = ps_ss.tile([1, BQ], F32, tag="ss")
            ppn = ps_ss.tile([1, BP], F32, tag="ss")
            for k in range(HC):
                nc.tensor.matmul(pd[:], qcls[k][:], pcls[k][:],
                                 start=(k == 0), stop=(k == HC - 1))
                qsq = fin.tile([128, BQ], F32R, tag="qsq")
                nc.scalar.activation(out=qsq[:], in_=qcls[k][:], func=AF.Square)
                nc.tensor.matmul(pqn[:], ones_c[:], qsq[:],
                                 start=(k == 0), stop=(k == HC - 1))
                psq = fin.tile([128, BP], F32R, tag="psq")
                nc.scalar.activation(out=psq[:], in_=pcls[k][:], func=AF.Square)
                nc.tensor.matmul(ppn[:], ones_c[:], psq[:],
                                 start=(k == 0), stop=(k == HC - 1))
            pdsb = fin.tile([BQ, BP], F32, tag="pdsb")
            nc.scalar.copy(out=pdsb[:], in_=pd[:])
            rq_row = fin.tile([1, BQ], F32, tag="rq_row")
            nc.scalar.activation(out=rq_row[:], in_=pqn[:], func=AF.Sqrt)
            nc.vector.tensor_scalar(out=rq_row[:], in0=rq_row[:], scalar1=1e-12,
                                    scalar2=None, op0=AX.max)
            nc.vector.reciprocal(out=rq_row[:], in_=rq_row[:])
            rp_row = fin.tile([1, BP], F32R, tag="rp_row")
            nc.scalar.activation(out=rp_row[:], in_=ppn[:], func=AF.Sqrt)
            nc.vector.tensor_scalar(out=rp_row[:], in0=rp_row[:], scalar1=1e-12,
                                    scalar2=None, op0=AX.max)
            nc.vector.reciprocal(out=rp_row[:], in_=rp_row[:])
            # rq as a column via DRAM bounce
            nc.sync.dma_start(out=d_rq[:], in_=rq_row[:])
            rq_col = fin.tile([BQ, 1], F32, tag="rq_col")
            nc.sync.dma_start(
                out=rq_col[:],
                in_=bass.AP(tensor=d_rq.tensor, offset=0, ap=[[1, BQ], [0, 1]]))
            # rp broadcast across 8 partitions
            prpb = ps_misc.tile([BQ, BP], F32, tag="misc")
            nc.tensor.matmul(prpb[:], ones_r[:, 0:BQ], rp_row[:],
                             start=True, stop=True)
            rpB = fin.tile([BQ, BP], F32, tag="rpB")
            nc.scalar.copy(out=rpB[:], in_=prpb[:])
            dmul = fin.tile([BQ, BP], F32, tag="dmul")
            nc.vector.tensor_mul(dmul[:], pdsb[:], rpB[:])
            dout = fin.tile([BQ, BP], F32, tag="dout")
            nc.vector.tensor_scalar(out=dout[:], in0=dmul[:], scalar1=rq_col[:],
                                    scalar2=1.0 / TEMP, op0=AX.mult, op1=AX.mult)
            nc.sync.dma_start(out=o_dense[:], in_=dout[:])


# ---------------------------------------------------------------------------
# Host-side driver
# ---------------------------------------------------------------------------
_PROGRAM = None


def _get_program():
    global _PROGRAM
    if _PROGRAM is None:
        _PROGRAM = _build_program()
    return _PROGRAM


def _prep_ids(ids, sentinel):
    f = ids.astype(np.float32)
    return np.where(ids <= 3, np.float32(sentinel), f).astype(np.float32)


def make_in_maps(q_hidden, p_hidden, q_mask, p_mask, q_ids, p_ids,
                 colbert_w, colbert_b, sparse_w, sparse_b):
    import ml_dtypes
    q_hidden = np.asarray(q_hidden, np.float32)
    p_hidden = np.asarray(p_hidden, np.float32)
    q_mask = np.ascontiguousarray(np.asarray(q_mask, np.float32))
    p_mask = np.ascontiguousarray(np.asarray(p_mask, np.float32))
    colbert_w = np.asarray(colbert_w, np.float32)
    colbert_b = np.ascontiguousarray(np.asarray(colbert_b, np.float32))
    sparse_w = np.ascontiguousarray(np.asarray(sparse_w, np.float32))
    sparse_b = np.asarray(sparse_b, np.float32).reshape(1, 1)
    q_ids = np.asarray(q_ids)
    p_ids = np.asarray(p_ids)
    qi = _prep_ids(q_ids, -2.0)
    ones_col = np.ones((128, 1), np.float32)
    ones_pair = np.ones((128, 2, 1), ml_dtypes.float8_e4m3)
    ones_row = np.ones((1, 128), np.float32)
    a = np.arange(128)
    lmask = (a[None, :] < a[:, None]).astype(np.float32)  # [a, a'] = a' < a

    bf16 = ml_dtypes.bfloat16
    f8 = ml_dtypes.float8_e4m3

    # Host-side layout transforms (pure data movement + dtype casts),
    # partition-major so each DMA is 128 contiguous descriptors:
    # q_hiddenT[p, k, i*LQ+l] = q_hidden[i, l, k*128+p]
    qhT = np.ascontiguousarray(
        q_hidden.transpose(2, 0, 1).reshape(HC, 128, BQ * LQ)
        .transpose(1, 0, 2).astype(bf16))
    # p_hiddenT[j, p, k, l] = p_hidden[j, l, k*128+p]
    phT = np.ascontiguousarray(
        p_hidden.transpose(0, 2, 1).reshape(BP_FULL, HC, 128, LP)
        .transpose(0, 2, 1, 3).astype(bf16))
    # fp8 DoubleRow pair layouts: hidden index h = (2g+t)*128+p
    # q_hidden8[p, g, t, i*LQ+l]
    q8 = np.ascontiguousarray(
        q_hidden.transpose(2, 0, 1).reshape(HC // 2, 2, 128, BQ * LQ)
        .transpose(2, 0, 1, 3).astype(f8))
    # p_hidden8[j, p, g, t, l]
    p8 = np.ascontiguousarray(
        p_hidden.transpose(0, 2, 1).reshape(BP_FULL, HC // 2, 2, 128, LP)
        .transpose(0, 3, 1, 2, 4).astype(f8))
    # colbert_w8[p, g, t, m] = W_SCALE * colbert_w[m, (2g+t)*128+p]
    w8 = np.ascontiguousarray(
        (colbert_w.T * W_SCALE).reshape(HC // 2, 2, 128, H)
        .transpose(2, 0, 1, 3).astype(f8))

    in_maps = []
    for c in range(N_CORES):
        sl = slice(c * BP, (c + 1) * BP)
        in_maps.append({
            "q_hiddenT": qhT,
            "p_hiddenT": phT[sl],
            "q_hidden8": q8,
            "p_hidden8": p8[sl],
            "colbert_w8": w8,
            "q_mask": q_mask,
            "p_mask": np.ascontiguousarray(p_mask[sl]),
            "q_ids_f": qi,
            "p_ids_f": np.ascontiguousarray(_prep_ids(p_ids[sl], -1.0)),
            "colbert_b": colbert_b,
            "sparse_w": sparse_w.astype(bf16),
            "sparse_b": sparse_b,
            "ones_col": ones_col,
            "ones_pair": ones_pair,
            "ones_row": ones_row,
            "lmask": lmask,
        })
    return in_maps


def kernel(q_hidden, p_hidden, q_mask, p_mask, q_ids, p_ids,
           colbert_w, colbert_b, sparse_w, sparse_b):
    nc = _get_program()
    in_maps = make_in_maps(q_hidden, p_hidden, q_mask, p_mask, q_ids, p_ids,
                           colbert_w, colbert_b, sparse_w, sparse_b)
    res = run_bass_kernel_spmd(nc, in_maps, list(range(N_CORES)))
    dense = np.concatenate([res.results[c]["dense"] for c in range(N_CORES)], axis=1)
    sparse = np.concatenate([res.results[c]["sparse"] for c in range(N_CORES)], axis=1)
    colbert = np.concatenate([res.results[c]["colbert"] for c in range(N_CORES)],
                             axis=1)
    return dense, sparse, colbert



# revision 44
# speedup vs baseline: 25.0395x; 25.0395x over previous
"""BGE-M3 scoring kernel for 8 Trainium2 NeuronCores.

Data-parallel over the 64 passages (8 per core); query side replicated.
Each core produces the [8, 8] column block of dense/sparse/colbert scores
for its passages; the host concatenates blocks along axis 1.

Self-contained: builds the Bass program once (module cache) and runs it
via run_bass_kernel_spmd on cores 0-7.
"""
import numpy as np
import concourse.bass as bass
import concourse.tile as tile
import concourse.mybir as mybir
from concourse.bass_utils import run_bass_kernel_spmd
from concourse.vector_clock import ScopedClock

F32 = mybir.dt.float32
F32R = mybir.dt.float32r
BF16 = mybir.dt.bfloat16
F8 = mybir.dt.float8e4
PM = mybir.MatmulPerfMode
AX = mybir.AluOpType
AF = mybir.ActivationFunctionType
X = mybir.AxisListType.X
FP8_SCALE = 16.0  # colbert unit vectors are scaled by this before fp8 cast
W_SCALE = 64.0    # colbert_w is scaled by this before fp8 cast (sigma 0.02)

N_CORES = 8
H = 1024
BQ, LQ = 8, 128
BP_FULL, LP = 64, 512
BP = BP_FULL // N_CORES          # 8 passages per core
HC = H // 128                    # 8 chunks of the hidden dim
TEMP = 0.02

# ---------------------------------------------------------------------------
# Walrus workaround: this container's neuronxcc rejects >1 sem wait per
# instruction ("Too many sync wait commands"). Split extra waits onto
# single-wait NOPs inserted just before the instruction on the same engine.
# ---------------------------------------------------------------------------
_wait_counter = [0]


def _split_multi_waits(nc):
    for fn in nc.m.functions:
        for bb in fn.blocks:
            out, changed = [], False
            for inst in bb.instructions:
                si = inst.sync_info
                if si is not None and len(si.on_wait) > 1:
                    changed = True
                    waits = list(si.on_wait)
                    for w in waits[:-1]:
                        _wait_counter[0] += 1
                        nop = mybir.InstNoOp(
                            name=f"I-waitsplit-{_wait_counter[0]}", ins=[], outs=[])
                        nop.engine = inst.engine
                        nop.sync_info = mybir.SyncInfo(on_wait=[w], on_update=[])
                        nc.register_instruction(nop)
                        out.append(nop)
                    inst.sync_info = mybir.SyncInfo(
                        on_wait=[waits[-1]], on_update=list(si.on_update))
                out.append(inst)
            if changed:
                bb.instructions = out


class _TC(tile.TileContext):
    def _drain_and_barrier(self, tick_clock, wait_clock):
        nc = self.nc
        drain_inst = nc.sync.drain()
        wait_clock.add_sem_waits(
            drain_inst.ins, ScopedClock({None: tick_clock.global_clock}))
        nc.all_engine_barrier()
        assert self.sems is not None
        popped = nc._tile_sem_poison_stack.pop()
        assert popped is self._sem_poison
        nc.clear_and_free_semaphores(list(self.sems.allocated().values()))
        nc.all_engine_barrier()

    def __exit__(self, *args):
        r = super().__exit__(*args)
        _split_multi_waits(self.nc)
        return r


def _bcast_rows(row_ap, parts=128):
    """DMA source AP replicating one DRAM row across `parts` partitions."""
    return bass.AP(tensor=row_ap.tensor, offset=row_ap.offset,
                   ap=[[0, parts]] + [list(d) for d in row_ap.ap])


# ---------------------------------------------------------------------------
# Program construction
# ---------------------------------------------------------------------------
def _build_program(repeats=1):
    nc = bass.Bass()

    # DRAM I/O (per core). Hidden states arrive host-pre-transposed
    # (chunk-major, hidden-on-partitions): bf16 full-precision copies for the
    # sparse/dense paths, fp8 pair layouts (DoubleRow) for the projections.
    d_qhT = nc.dram_tensor("q_hiddenT", [128, HC, BQ * LQ], BF16,
                           kind="ExternalInput")
    d_phT = nc.dram_tensor("p_hiddenT", [BP, 128, HC, LP], BF16,
                           kind="ExternalInput")
    d_q8 = nc.dram_tensor("q_hidden8", [128, HC // 2, 2, BQ * LQ], F8,
                          kind="ExternalInput")
    d_p8 = nc.dram_tensor("p_hidden8", [BP, 128, HC // 2, 2, LP], F8,
                          kind="ExternalInput")
    d_w8 = nc.dram_tensor("colbert_w8", [128, HC // 2, 2, H], F8,
                          kind="ExternalInput")
    d_qm = nc.dram_tensor("q_mask", [BQ, LQ], F32R, kind="ExternalInput")
    d_pm = nc.dram_tensor("p_mask", [BP, LP], F32R, kind="ExternalInput")
    d_qi = nc.dram_tensor("q_ids_f", [BQ, LQ], F32, kind="ExternalInput")
    d_pi = nc.dram_tensor("p_ids_f", [BP, LP], F32, kind="ExternalInput")
    d_cb = nc.dram_tensor("colbert_b", [H], F32, kind="ExternalInput")
    d_sw = nc.dram_tensor("sparse_w", [H], BF16, kind="ExternalInput")
    d_sb = nc.dram_tensor("sparse_b", [1, 1], F32, kind="ExternalInput")
    d_oc = nc.dram_tensor("ones_col", [128, 1], F32R, kind="ExternalInput")
    d_op = nc.dram_tensor("ones_pair", [128, 2, 1], F8, kind="ExternalInput")
    d_or = nc.dram_tensor("ones_row", [1, 128], F32R, kind="ExternalInput")
    d_lm = nc.dram_tensor("lmask", [128, 128], F32, kind="ExternalInput")

    o_dense = nc.dram_tensor("dense", [BQ, BP], F32, kind="ExternalOutput")
    o_sparse = nc.dram_tensor("sparse", [BQ, BP], F32, kind="ExternalOutput")
    o_colbert = nc.dram_tensor("colbert", [BQ, BP], F32, kind="ExternalOutput")

    with _TC(nc) as tc:
        for _ in range(repeats):
            _emit(nc, tc, d_qhT, d_phT, d_q8, d_p8, d_w8, d_qm, d_pm, d_qi,
                  d_pi, d_cb, d_sw, d_sb, d_oc, d_op, d_or, d_lm, o_dense,
                  o_sparse, o_colbert)
    return nc


def _emit(nc, tc, d_qhT, d_phT, d_q8, d_p8, d_w8, d_qm, d_pm, d_qi, d_pi,
          d_cb, d_sw, d_sb, d_oc, d_op, d_or, d_lm, o_dense, o_sparse,
          o_colbert):
    from contextlib import ExitStack
    es = ExitStack()
    with es:
        es.enter_context(nc.allow_low_precision(reason="fp22/f32r is the target precision"))
        # ---- pools -------------------------------------------------------
        persist = es.enter_context(tc.tile_pool(name="persist", bufs=1))
        wt_pool = es.enter_context(tc.tile_pool(name="wt", bufs=1))
        qcolT_pool = es.enter_context(tc.tile_pool(name="qcolT", bufs=1))
        dram = es.enter_context(tc.tile_pool(name="dram", bufs=1, space="DRAM"))
        ps_mm = es.enter_context(tc.tile_pool(name="ps_mm", bufs=3, space="PSUM"))
        ps_ss = es.enter_context(tc.tile_pool(name="ps_ss", bufs=2, space="PSUM"))
        ps_misc = es.enter_context(tc.tile_pool(name="ps_misc", bufs=1, space="PSUM"))

        # ---- persistent small tiles --------------------------------------
        ones_c = persist.tile([128, 1], F32R, tag="ones_c")
        nc.sync.dma_start(out=ones_c[:], in_=d_oc[:])
        ones_p = persist.tile([128, 2, 1], F8, tag="ones_p")
        nc.sync.dma_start(out=ones_p[:], in_=d_op[:])
        ones_r = persist.tile([1, 128], F32R, tag="ones_r")
        nc.sync.dma_start(out=ones_r[:], in_=d_or[:])
        lmask = persist.tile([128, 128], F32, tag="lmask")
        nc.sync.dma_start(out=lmask[:], in_=d_lm[:])
        sb_sb = persist.tile([1, 1], F32, tag="sb")
        nc.sync.dma_start(out=sb_sb[:], in_=d_sb[:])
        # colbert bias chunks: cb_sb[:, m] = b[m*128:(m+1)*128]
        cb_sb = persist.tile([128, HC], F32, tag="cb")
        nc.sync.dma_start(out=cb_sb[:], in_=d_cb.ap().rearrange("(m p) -> p m", p=128))
        # sparse_w chunks as lhsT columns: sw_sb[:, k] = sw[k*128:(k+1)*128]
        sw_sb = persist.tile([128, HC], BF16, tag="sw")
        nc.sync.dma_start(out=sw_sb[:], in_=d_sw.ap().rearrange("(k p) -> p k", p=128))
        # q ids as per-token columns: qid_cols[:, i] = q_ids_f[i, :]
        qid_cols = persist.tile([128, BQ], F32, tag="qid_cols")
        nc.sync.dma_start(out=qid_cols[:], in_=d_qi.ap().rearrange("i l -> l i"))
        # q mask tokens 1..127 transposed [token-1, batch] for qlen
        qmT = persist.tile([128, BQ], F32R, tag="qmT")
        nc.sync.dma_start(
            out=qmT[0:127, :],
            in_=bass.AP(tensor=d_qm.ap().tensor, offset=1,
                        ap=[[1, 127], [128, BQ]]))

        qcls_t = persist.tile([128, HC, BQ], BF16, tag="qcls")
        qcls = [qcls_t[:, k, :] for k in range(HC)]
        pcls_t = persist.tile([128, HC, BP], BF16, tag="pcls")
        pcls = [pcls_t[:, k, :] for k in range(HC)]
        rmax = [persist.tile([128, BP], F32R, tag=f"rmax{i}", name=f"rmax{i}")
                for i in range(BQ)]
        smax = [persist.tile([128, BP], F32R, tag=f"smax{i}", name=f"smax{i}")
                for i in range(BQ)]
        qw = [persist.tile([128, 1], F32R, tag=f"qw{i}", name=f"qw{i}")
              for i in range(BQ)]
        # per-(token,query) colbert final scale columns
        qnr = persist.tile([128, BQ], F32R, tag="qnr")
        twq_cols = persist.tile([128, BQ], F32R, tag="twq_cols")

        # fp8 pair-layout W (x W_SCALE): w8[g][:, t, m] = W[m, (2g+t)*128+p]
        w8_t = wt_pool.tile([128, HC // 2, 2, H], F8, tag="w8")
        w8 = [w8_t[:, g, :, :] for g in range(HC // 2)]
        # fp8 pair-layout q colbert vectors: qcolT8[g][:, t, :] holds hidden
        # chunk m=2g+t, scaled by FP8_SCALE (DoubleRow contracts 2x128=256)
        qcolT8 = [qcolT_pool.tile([128, 2, BQ * LQ], F8, tag=f"qct{g}",
                                  name=f"qct{g}") for g in range(HC // 2)]

        d_twq = dram.tile([1, BQ * LQ], F32R, name="d_twq")
        d_qnr = dram.tile([1, BQ * LQ], F32R, name="d_qnr")
        d_rq = dram.tile([1, BQ], F32, name="d_rq")

        # ================= SETUP: W8 (host-pre-transposed fp8 pairs) ======
        nc.sync.dma_start(out=w8_t[:], in_=d_w8.ap())

        # ================= SETUP: q side ==================================
        with tc.tile_pool(name="qhT", bufs=1) as qhT_pool, \
             tc.tile_pool(name="qtmp", bufs=1) as qtmp_pool, \
             tc.tile_pool(name="qv", bufs=2) as qv_pool:
            qhT_t = qhT_pool.tile([128, HC, BQ * 128], BF16, tag="qhT")
            nc.sync.dma_start(out=qhT_t[:], in_=d_qhT.ap())
            qhT = [qhT_t[:, k, :] for k in range(HC)]
            q8_t = qhT_pool.tile([128, HC // 2, 2, BQ * LQ], F8, tag="q8")
            nc.scalar.dma_start(out=q8_t[:], in_=d_q8.ap())
            q8 = [q8_t[:, g, :, :] for g in range(HC // 2)]
            # CLS columns (token 0 of each batch), all chunks in one copy
            nc.scalar.copy(
                out=qcls_t[:],
                in_=qhT_t.rearrange("p k (i l) -> p k i l", i=BQ)[:, :, :, 0])

            # token weights tw_q = relu(qh . sw + b), all 128 tokens per batch
            twq_row = qtmp_pool.tile([1, BQ * 128], F32R, tag="twq")
            for g in range(2):
                ptw = ps_ss.tile([1, 512], F32, tag="ss")
                for k in range(HC):
                    nc.tensor.matmul(ptw[:], sw_sb[:, k:k + 1],
                                     qhT[k][:, g * 512:(g + 1) * 512],
                                     start=(k == 0), stop=(k == HC - 1))
                nc.scalar.activation(out=twq_row[:, g * 512:(g + 1) * 512],
                                     in_=ptw[:], func=AF.Relu, bias=sb_sb[:], scale=1.0)
            # column form via DRAM bounce: twq_cols[l, i] = tw_q[i, l]
            nc.sync.dma_start(out=d_twq[:], in_=twq_row[:])
            nc.sync.dma_start(
                out=twq_cols[:],
                in_=bass.AP(tensor=d_twq.tensor, offset=0, ap=[[1, 128], [128, BQ]]))

            # ---- q dedup: keep, per (batch, id), the max-tw token --------
            for i in range(BQ):
                pb = ps_misc.tile([128, 128], F32, tag="misc")
                nc.tensor.matmul(pb[:], ones_r[:],
                                 twq_row[:, i * 128:(i + 1) * 128],
                                 start=True, stop=True)
                twB = qtmp_pool.tile([128, 128], F32, tag="twB")
                nc.scalar.copy(out=twB[:], in_=pb[:])
                tw_col = twq_cols[:, i:i + 1]
                qidB = qtmp_pool.tile([128, 128], F32, tag="qidB")
                nc.gpsimd.dma_start(out=qidB[:],
                                    in_=_bcast_rows(d_qi[i:i + 1, :]))
                # dominated(a) = sum_a' eq_id*(gt_tw + eq_tw*lower)
                x2 = qtmp_pool.tile([128, 128], F32, tag="x2")
                nc.vector.scalar_tensor_tensor(
                    out=x2[:], in0=twB[:], scalar=tw_col, in1=lmask[:],
                    op0=AX.is_equal, op1=AX.mult)
                x3 = qtmp_pool.tile([128, 128], F32, tag="x3")
                nc.vector.scalar_tensor_tensor(
                    out=x3[:], in0=twB[:], scalar=tw_col, in1=x2[:],
                    op0=AX.is_gt, op1=AX.add)
                dom = qtmp_pool.tile([128, 128], F32, tag="dom")
                dsum = qtmp_pool.tile([128, 1], F32, tag="dsum")
                nc.vector.scalar_tensor_tensor(
                    out=dom[:], in0=qidB[:], scalar=qid_cols[:, i:i + 1], in1=x3[:],
                    op0=AX.is_equal, op1=AX.mult, accum_out=dsum[:])
                surv = qtmp_pool.tile([128, 1], F32, tag="surv")
                nc.vector.tensor_scalar(out=surv[:], in0=dsum[:], scalar1=0.0,
                                        scalar2=None, op0=AX.is_equal)
                nc.vector.tensor_mul(qw[i][:], surv[:], tw_col)

            # ---- q colbert projection: raw v cast straight to fp8 pairs;
            # per-token 1/norm goes into the qnr finals column instead.
            qn_row = qtmp_pool.tile([1, BQ * 128], F32, tag="qn_row")
            for g in range(2):
                vsq8 = [qv_pool.tile([128, 2, 512], F8, tag=f"qvs{gg}",
                                     name=f"qvs{gg}") for gg in range(HC // 2)]
                pss = ps_ss.tile([1, 512], F32, tag="ss")
                for m in range(HC):
                    pmm = ps_mm.tile([128, 512], F32, tag="mm")
                    for kg in range(HC // 2):
                        nc.tensor.matmul(pmm[:],
                                         w8[kg][:, :, m * 128:(m + 1) * 128],
                                         q8[kg][:, :, g * 512:(g + 1) * 512],
                                         start=(kg == 0), stop=(kg == HC // 2 - 1),
                                         perf_mode=PM.DoubleRow)
                    nc.scalar.activation(
                        out=qcolT8[m // 2][:, m % 2, g * 512:(g + 1) * 512],
                        in_=pmm[:], func=AF.Identity,
                        bias=cb_sb[:, m:m + 1], scale=1.0 / W_SCALE)
                    nc.scalar.activation(out=vsq8[m // 2][:, m % 2, :], in_=pmm[:],
                                         func=AF.Square,
                                         bias=cb_sb[:, m:m + 1], scale=1.0 / W_SCALE)
                for gg in range(HC // 2):
                    for t in range(2):
                        nc.tensor.matmul(pss[:], ones_p[:, 0, :],
                                         vsq8[gg][:, t, :],
                                         start=(gg == 0 and t == 0),
                                         stop=(gg == HC // 2 - 1 and t == 1))
                nc.scalar.activation(out=qn_row[:, g * 512:(g + 1) * 512],
                                     in_=pss[:], func=AF.Sqrt)
            # qnr0 = qmask / ||v|| as a [1, BQ*LQ] row
            rq_full = qtmp_pool.tile([1, BQ * 128], F32R, tag="rq_full")
            nc.vector.reciprocal(out=rq_full[:], in_=qn_row[:])
            mrow = qtmp_pool.tile([1, BQ * 128], F32R, tag="mrow")
            nc.sync.dma_start(
                out=mrow[:],
                in_=bass.AP(tensor=d_qm.ap().tensor, offset=0,
                            ap=[[0, 1], [1, BQ * 128]]))
            nc.vector.tensor_mul(rq_full[:], rq_full[:], mrow[:])

            # qnr[a-1, i] = qmask[i,a] / (||v_q[i,a]|| * qlen_i * TEMP)
            # (tokens 1..127 on partitions 0..126, via DRAM bounce)
            nc.sync.dma_start(out=d_qnr[:], in_=rq_full[:])
            nc.sync.dma_start(
                out=qnr[0:127, :],
                in_=bass.AP(tensor=d_qnr.tensor, offset=1,
                            ap=[[1, 127], [128, BQ]]))
            pql = ps_ss.tile([1, BQ], F32, tag="ss")
            nc.tensor.matmul(pql[:], ones_c[0:127, :], qmT[0:127, :],
                             start=True, stop=True)
            qiv_row = qtmp_pool.tile([1, BQ], F32R, tag="qiv")
            nc.vector.tensor_scalar(out=qiv_row[:], in0=pql[:],
                                    scalar1=TEMP * FP8_SCALE,
                                    scalar2=None, op0=AX.mult)
            nc.vector.reciprocal(out=qiv_row[:], in_=qiv_row[:])
            pqb = ps_misc.tile([127, BQ], F32, tag="misc")
            nc.tensor.matmul(pqb[:], ones_r[:, 0:127], qiv_row[:],
                             start=True, stop=True)
            qivB = qtmp_pool.tile([128, BQ], F32, tag="qivB")
            nc.scalar.copy(out=qivB[0:127, :], in_=pqb[:])
            nc.vector.tensor_mul(qnr[0:127, :], qnr[0:127, :], qivB[0:127, :])

        # ================= MAIN LOOP over passages ========================
        # Software-pipelined: stage A(j) = DMA+transpose+project+normalize
        # (produces pcolT_j, twpB_j, pidB_j), stage B(j) = colbert scores +
        # sparse match. Emission order A0, A1, B0, A2, B1, ... keeps the PE
        # stream free of stalls: B(j)'s operands are ready by the time the
        # in-order PE queue reaches them.
        with tc.tile_pool(name="phT", bufs=2) as phT_pool, \
             tc.tile_pool(name="pcs", bufs=2) as pcs_pool, \
             tc.tile_pool(name="pvq", bufs=2) as pvq_pool, \
             tc.tile_pool(name="pcolT", bufs=2) as pcolT_pool, \
             tc.tile_pool(name="prow2", bufs=2) as prow2_pool, \
             tc.tile_pool(name="prow", bufs=1) as prow_pool:

            def stage_a(j):
                phT_t = phT_pool.tile([128, HC, LP], BF16, tag="phT")
                nc.sync.dma_start(out=phT_t[:], in_=d_phT[j])
                phT = [phT_t[:, k, :] for k in range(HC)]
                p8 = phT_pool.tile([128, HC // 2, 2, LP], F8, tag="p8")
                nc.sync.dma_start(out=p8[:], in_=d_p8[j])
                nc.scalar.copy(out=pcls_t[:, :, j], in_=phT_t[:, :, 0])

                # token weights tw_p = relu(ph . sw + b)
                ptw = ps_ss.tile([1, LP], F32, tag="ss")
                for k in range(HC):
                    nc.tensor.matmul(ptw[:], sw_sb[:, k:k + 1], phT[k][:],
                                     start=(k == 0), stop=(k == HC - 1))
                twp_row = prow_pool.tile([1, LP], F32R, tag="twp")
                nc.scalar.activation(out=twp_row[:], in_=ptw[:], func=AF.Relu,
                                     bias=sb_sb[:], scale=1.0)
                ptb = ps_misc.tile([128, LP], F32, tag="misc")
                nc.tensor.matmul(ptb[:], ones_r[:], twp_row[:], start=True, stop=True)
                twpB = prow2_pool.tile([128, LP], F32, tag="twpB")
                nc.scalar.copy(out=twpB[:], in_=ptb[:])
                pidB = prow2_pool.tile([128, LP], F32, tag="pidB")
                nc.gpsimd.dma_start(out=pidB[:], in_=_bcast_rows(d_pi[j:j + 1, :]))

                # colbert projection (all 512 tokens): raw v -> fp8 pairs
                pcolT8 = [pcolT_pool.tile([128, 2, LP], F8, tag=f"pct{g}",
                                          name=f"pct{g}_{j}")
                          for g in range(HC // 2)]
                vsq8 = [pvq_pool.tile([128, 2, LP], F8, tag=f"pvs{gg}",
                                      name=f"pvs{gg}") for gg in range(HC // 2)]
                pss = ps_ss.tile([1, LP], F32, tag="ss")
                for m in range(HC):
                    pmm = ps_mm.tile([128, LP], F32, tag="mm")
                    for kg in range(HC // 2):
                        nc.tensor.matmul(pmm[:],
                                         w8[kg][:, :, m * 128:(m + 1) * 128],
                                         p8[:, kg, :, :],
                                         start=(kg == 0), stop=(kg == HC // 2 - 1),
                                         perf_mode=PM.DoubleRow)
                    nc.scalar.activation(out=pcolT8[m // 2][:, m % 2, :],
                                         in_=pmm[:], func=AF.Identity,
                                         bias=cb_sb[:, m:m + 1], scale=1.0 / W_SCALE)
                    nc.scalar.activation(out=vsq8[m // 2][:, m % 2, :], in_=pmm[:],
                                         func=AF.Square,
                                         bias=cb_sb[:, m:m + 1], scale=1.0 / W_SCALE)
                for gg in range(HC // 2):
                    for t in range(2):
                        nc.tensor.matmul(pss[:], ones_p[:, 0, :],
                                         vsq8[gg][:, t, :],
                                         start=(gg == 0 and t == 0),
                                         stop=(gg == HC // 2 - 1 and t == 1))
                nrow = prow_pool.tile([1, LP], F32, tag="nrow")
                nc.scalar.activation(out=nrow[:], in_=pss[:], func=AF.Sqrt,
                                     scale=1.0 / (FP8_SCALE * FP8_SCALE))
                rrow = prow_pool.tile([1, LP], F32, tag="rrow")
                nc.vector.reciprocal(out=rrow[:], in_=nrow[:])
                rp_row = prow_pool.tile([1, LP], F32R, tag="rp_row")
                mrow = prow_pool.tile([1, LP], F32R, tag="mrow")
                nc.scalar.dma_start(out=mrow[:], in_=d_pm[j:j + 1, :])
                nc.vector.tensor_mul(rp_row[:], rrow[:], mrow[:])
                pbc = ps_misc.tile([128, LP], F32, tag="misc")
                nc.tensor.matmul(pbc[:], ones_r[:], rp_row[:], start=True, stop=True)
                rpB = prow2_pool.tile([128, LP], F32, tag="rpB")
                nc.scalar.copy(out=rpB[:], in_=pbc[:])
                # normalize on Pool: pcolT8s = fp8(v8 * FP8_SCALE*mask/||v||)
                pcolT8s = [pcs_pool.tile([128, 2, LP], F8, tag=f"pcs{g}",
                                         name=f"pcs{g}_{j}")
                           for g in range(HC // 2)]
                for g in range(HC // 2):
                    for t in range(2):
                        nc.vector.tensor_mul(pcolT8s[g][:, t, :],
                                             pcolT8[g][:, t, :], rpB[:])
                return pcolT8s, twpB, pidB

            def stage_b(j, st):
                pcolT8s, twpB, pidB = st
                for i in range(BQ):
                    psc = ps_mm.tile([127, LP], F32, tag="mm")
                    for g in range(HC // 2):
                        nc.tensor.matmul(
                            psc[:],
                            qcolT8[g][:, :, i * 128 + 1:(i + 1) * 128],
                            pcolT8s[g][:, :, :],
                            start=(g == 0), stop=(g == HC // 2 - 1),
                            perf_mode=PM.DoubleRow)
                    nc.vector.reduce_max(out=rmax[i][0:127, j:j + 1],
                                         in_=psc[:, 1:LP], axis=X)
                    mt = prow_pool.tile([128, LP], F32, tag="mt")
                    nc.vector.scalar_tensor_tensor(
                        out=mt[:], in0=pidB[:], scalar=qid_cols[:, i:i + 1],
                        in1=twpB[:], op0=AX.is_equal, op1=AX.mult)
                    nc.vector.reduce_max(out=smax[i][:, j:j + 1], in_=mt[:], axis=X)

            pending = stage_a(0)
            for j in range(1, BP):
                nxt = stage_a(j)
                stage_b(j - 1, pending)
                pending = nxt
            stage_b(BP - 1, pending)

        # ================= FINALS =========================================
        with tc.tile_pool(name="fin", bufs=1) as fin:
            for i in range(BQ):
                pcbi = ps_ss.tile([1, BP], F32, tag="ss")
                nc.tensor.matmul(pcbi[:], qnr[0:127, i:i + 1],
                                 rmax[i][0:127, :], start=True, stop=True)
                stag = fin.tile([1, BP], F32, tag=f"cst{i}", name=f"cst{i}")
                nc.scalar.copy(out=stag[:], in_=pcbi[:])
                nc.sync.dma_start(out=o_colbert[i:i + 1, :], in_=stag[:])

                pspi = ps_ss.tile([1, BP], F32, tag="ss")
                nc.tensor.matmul(pspi[:], qw[i][:], smax[i][:],
                                 start=True, stop=True)
                stag2 = fin.tile([1, BP], F32, tag=f"sst{i}", name=f"sst{i}")
                nc.scalar.activation(out=stag2[:], in_=pspi[:], func=AF.Copy,
                                     scale=1.0 / TEMP)
                nc.sync.dma_start(out=o_sparse[i:i + 1, :], in_=stag2[:])

            # dense scores
            pd = ps_misc.tile([BQ, BP], F32, tag="misc")
            pqn = ps_ss.tile([1, BQ], F32, tag="ss")
            ppn = ps_ss.tile([1, BP], F32, tag="ss")
            for k in range(HC):
                nc.tensor.matmul(pd[:], qcls[k][:], pcls[k][:],
                                 start=(k == 0), stop=(k == HC - 1))
                qsq = fin.tile([128, BQ], F32R, tag="qsq")
                nc.scalar.activation(out=qsq[:], in_=qcls[k][:], func=AF.Square)
                nc.tensor.matmul(pqn[:], ones_c[:], qsq[:],
                                 start=(k == 0), stop=(k == HC - 1))
                psq = fin.tile([128, BP], F32R, tag="psq")
                nc.scalar.activation(out=psq[:], in_=pcls[k][:], func=AF.Square)
                nc.tensor.matmul(ppn[:], ones_c[:], psq[:],
                                 start=(k == 0), stop=(k == HC - 1))
            pdsb = fin.tile([BQ, BP], F32, tag="pdsb")
            nc.scalar.copy(out=pdsb[:], in_=pd[:])
            rq_row = fin.tile([1, BQ], F32, tag="rq_row")
            nc.scalar.activation(out=rq_row[:], in_=pqn[:], func=AF.Sqrt)
            nc.vector.tensor_scalar(out=rq_row[:], in0=rq_row[:], scalar1=1e-12,
                                    scalar2=None, op0=AX.max)
            nc.vector.reciprocal(out=rq_row[:], in_=rq_row[:])
            rp_row = fin.tile([1, BP], F32R, tag="rp_row")
            nc.scalar.activation(out=rp_row[:], in_=ppn[:], func=AF.Sqrt)
            nc.vector.tensor_scalar(out=rp_row[:], in0=rp_row[:], scalar1=1e-12,
                                    scalar2=None, op0=AX.max)
            nc.vector.reciprocal(out=rp_row[:], in_=rp_row[:])
            # rq as a column via DRAM bounce
            nc.sync.dma_start(out=d_rq[:], in_=rq_row[:])
            rq_col = fin.tile([BQ, 1], F32, tag="rq_col")
            nc.sync.dma_start(
                out=rq_col[:],
                in_=bass.AP(tensor=d_rq.tensor, offset=0, ap=[[1, BQ], [0, 1]]))
            # rp broadcast across 8 partitions
            prpb = ps_misc.tile([BQ, BP], F32, tag="misc")
            nc.tensor.matmul(prpb[:], ones_r[:, 0:BQ], rp_row[:],
                             start=True, stop=True)
            rpB = fin.tile([BQ, BP], F32, tag="rpB")
            nc.scalar.copy(out=rpB[:], in_=prpb[:])
            dmul = fin.tile([BQ, BP], F32, tag="dmul")
            nc.vector.tensor_mul(dmul[:], pdsb[:], rpB[:])
            dout = fin.tile([BQ, BP], F32, tag="dout")
            nc.vector.tensor_scalar(out=dout[:], in0=dmul[:], scalar1=rq_col[:],
                                    scalar2=1.0 / TEMP, op0=AX.mult, op1=AX.mult)
            nc.sync.dma_start(out=o_dense[:], in_=dout[:])


# ---------------------------------------------------------------------------
# Host-side driver
# ---------------------------------------------------------------------------
_PROGRAM = None


def _get_program():
    global _PROGRAM
    if _PROGRAM is None:
        _PROGRAM = _build_program()
    return _PROGRAM


def _prep_ids(ids, sentinel):
    f = ids.astype(np.float32)
    return np.where(ids <= 3, np.float32(sentinel), f).astype(np.float32)


def make_in_maps(q_hidden, p_hidden, q_mask, p_mask, q_ids, p_ids,
                 colbert_w, colbert_b, sparse_w, sparse_b):
    import ml_dtypes
    q_hidden = np.asarray(q_hidden, np.float32)
    p_hidden = np.asarray(p_hidden, np.float32)
    q_mask = np.ascontiguousarray(np.asarray(q_mask, np.float32))
    p_mask = np.ascontiguousarray(np.asarray(p_mask, np.float32))
    colbert_w = np.asarray(colbert_w, np.float32)
    colbert_b = np.ascontiguousarray(np.asarray(colbert_b, np.float32))
    sparse_w = np.ascontiguousarray(np.asarray(sparse_w, np.float32))
    sparse_b = np.asarray(sparse_b, np.float32).reshape(1, 1)
    q_ids = np.asarray(q_ids)
    p_ids = np.asarray(p_ids)
    qi = _prep_ids(q_ids, -2.0)
    ones_col = np.ones((128, 1), np.float32)
    ones_pair = np.ones((128, 2, 1), ml_dtypes.float8_e4m3)
    ones_row = np.ones((1, 128), np.float32)
    a = np.arange(128)
    lmask = (a[None, :] < a[:, None]).astype(np.float32)  # [a, a'] = a' < a

    bf16 = ml_dtypes.bfloat16
    f8 = ml_dtypes.float8_e4m3

    # Host-side layout transforms (pure data movement + dtype casts),
    # partition-major so each DMA is 128 contiguous descriptors:
    # q_hiddenT[p, k, i*LQ+l] = q_hidden[i, l, k*128+p]
    qhT = np.ascontiguousarray(
        q_hidden.transpose(2, 0, 1).reshape(HC, 128, BQ * LQ)
        .transpose(1, 0, 2).astype(bf16))
    # p_hiddenT[j, p, k, l] = p_hidden[j, l, k*128+p]
    phT = np.ascontiguousarray(
        p_hidden.transpose(0, 2, 1).reshape(BP_FULL, HC, 128, LP)
        .transpose(0, 2, 1, 3).astype(bf16))
    # fp8 DoubleRow pair layouts: hidden index h = (2g+t)*128+p
    # q_hidden8[p, g, t, i*LQ+l]
    q8 = np.ascontiguousarray(
        q_hidden.transpose(2, 0, 1).reshape(HC // 2, 2, 128, BQ * LQ)
        .transpose(2, 0, 1, 3).astype(f8))
    # p_hidden8[j, p, g, t, l]
    p8 = np.ascontiguousarray(
        p_hidden.transpose(0, 2, 1).reshape(BP_FULL, HC // 2, 2, 128, LP)
        .transpose(0, 3, 1, 2, 4).astype(f8))
    # colbert_w8[p, g, t, m] = W_SCALE * colbert_w[m, (2g+t)*128+p]
    w8 = np.ascontiguousarray(
        (colbert_w.T * W_SCALE).reshape(HC // 2, 2, 128, H)
        .transpose(2, 0, 1, 3).astype(f8))

    in_maps = []
    for c in range(N_CORES):
        sl = slice(c * BP, (c + 1) * BP)
        in_maps.append({
            "q_hiddenT": qhT,
            "p_hiddenT": phT[sl],
            "q_hidden8": q8,
            "p_hidden8": p8[sl],
            "colbert_w8": w8,
            "q_mask": q_mask,
            "p_mask": np.ascontiguousarray(p_mask[sl]),
            "q_ids_f": qi,
            "p_ids_f": np.ascontiguousarray(_prep_ids(p_ids[sl], -1.0)),
            "colbert_b": colbert_b,
            "sparse_w": sparse_w.astype(bf16),
            "sparse_b": sparse_b,
            "ones_col": ones_col,
            "ones_pair": ones_pair,
            "ones_row": ones_row,
            "lmask": lmask,
        })
    return in_maps


def kernel(q_hidden, p_hidden, q_mask, p_mask, q_ids, p_ids,
           colbert_w, colbert_b, sparse_w, sparse_b):
    nc = _get_program()
    in_maps = make_in_maps(q_hidden, p_hidden, q_mask, p_mask, q_ids, p_ids,
                           colbert_w, colbert_b, sparse_w, sparse_b)
    res = run_bass_kernel_spmd(nc, in_maps, list(range(N_CORES)))
    dense = np.concatenate([res.results[c]["dense"] for c in range(N_CORES)], axis=1)
    sparse = np.concatenate([res.results[c]["sparse"] for c in range(N_CORES)], axis=1)
    colbert = np.concatenate([res.results[c]["colbert"] for c in range(N_CORES)],
                             axis=1)
    return dense, sparse, colbert



# revision 46
# speedup vs baseline: 962.9834x; 38.4586x over previous
"""BGE-M3 scoring kernel for 8 Trainium2 NeuronCores.

Data-parallel over the 64 passages (8 per core); query side replicated.
Each core produces the [8, 8] column block of dense/sparse/colbert scores
for its passages; the host concatenates blocks along axis 1.

Self-contained: builds the Bass program once (module cache) and runs it
via run_bass_kernel_spmd on cores 0-7.
"""
import numpy as np
import concourse.bass as bass
import concourse.tile as tile
import concourse.mybir as mybir
from concourse.bass_utils import run_bass_kernel_spmd
from concourse.vector_clock import ScopedClock

F32 = mybir.dt.float32
F32R = mybir.dt.float32r
BF16 = mybir.dt.bfloat16
F8 = mybir.dt.float8e4
PM = mybir.MatmulPerfMode
AX = mybir.AluOpType
AF = mybir.ActivationFunctionType
X = mybir.AxisListType.X
FP8_SCALE = 16.0  # colbert unit vectors are scaled by this before fp8 cast
W_SCALE = 64.0    # colbert_w is scaled by this before fp8 cast (sigma 0.02)

N_CORES = 8
H = 1024
BQ, LQ = 8, 128
BP_FULL, LP = 64, 512
BP = BP_FULL // N_CORES          # 8 passages per core
HC = H // 128                    # 8 chunks of the hidden dim
TEMP = 0.02

# ---------------------------------------------------------------------------
# Walrus workaround: this container's neuronxcc rejects >1 sem wait per
# instruction ("Too many sync wait commands"). Split extra waits onto
# single-wait NOPs inserted just before the instruction on the same engine.
# ---------------------------------------------------------------------------
_wait_counter = [0]


def _split_multi_waits(nc):
    for fn in nc.m.functions:
        for bb in fn.blocks:
            out, changed = [], False
            for inst in bb.instructions:
                si = inst.sync_info
                if si is not None and len(si.on_wait) > 1:
                    changed = True
                    waits = list(si.on_wait)
                    for w in waits[:-1]:
                        _wait_counter[0] += 1
                        nop = mybir.InstNoOp(
                            name=f"I-waitsplit-{_wait_counter[0]}", ins=[], outs=[])
                        nop.engine = inst.engine
                        nop.sync_info = mybir.SyncInfo(on_wait=[w], on_update=[])
                        nc.register_instruction(nop)
                        out.append(nop)
                    inst.sync_info = mybir.SyncInfo(
                        on_wait=[waits[-1]], on_update=list(si.on_update))
                out.append(inst)
            if changed:
                bb.instructions = out


class _TC(tile.TileContext):
    def _drain_and_barrier(self, tick_clock, wait_clock):
        nc = self.nc
        drain_inst = nc.sync.drain()
        wait_clock.add_sem_waits(
            drain_inst.ins, ScopedClock({None: tick_clock.global_clock}))
        nc.all_engine_barrier()
        assert self.sems is not None
        popped = nc._tile_sem_poison_stack.pop()
        assert popped is self._sem_poison
        nc.clear_and_free_semaphores(list(self.sems.allocated().values()))
        nc.all_engine_barrier()

    def __exit__(self, *args):
        r = super().__exit__(*args)
        _split_multi_waits(self.nc)
        return r


def _bcast_rows(row_ap, parts=128):
    """DMA source AP replicating one DRAM row across `parts` partitions."""
    return bass.AP(tensor=row_ap.tensor, offset=row_ap.offset,
                   ap=[[0, parts]] + [list(d) for d in row_ap.ap])


# ---------------------------------------------------------------------------
# Program construction
# ---------------------------------------------------------------------------
def _build_program(repeats=1):
    nc = bass.Bass()

    # DRAM I/O (per core). Hidden states arrive host-pre-transposed
    # (chunk-major, hidden-on-partitions): bf16 full-precision copies for the
    # sparse/dense paths, fp8 pair layouts (DoubleRow) for the projections.
    d_qhT = nc.dram_tensor("q_hiddenT", [128, HC, BQ * LQ], BF16,
                           kind="ExternalInput")
    d_phT = nc.dram_tensor("p_hiddenT", [BP, 128, HC, LP], BF16,
                           kind="ExternalInput")
    d_q8 = nc.dram_tensor("q_hidden8", [128, HC // 2, 2, BQ * LQ], F8,
                          kind="ExternalInput")
    d_p8 = nc.dram_tensor("p_hidden8", [BP, 128, HC // 2, 2, LP], F8,
                          kind="ExternalInput")
    d_w8 = nc.dram_tensor("colbert_w8", [128, HC // 2, 2, H], F8,
                          kind="ExternalInput")
    d_qm = nc.dram_tensor("q_mask", [BQ, LQ], F32R, kind="ExternalInput")
    d_pm = nc.dram_tensor("p_mask", [BP, LP], F32R, kind="ExternalInput")
    d_qi = nc.dram_tensor("q_ids_f", [BQ, LQ], F32, kind="ExternalInput")
    d_pi = nc.dram_tensor("p_ids_f", [BP, LP], F32, kind="ExternalInput")
    d_cb = nc.dram_tensor("colbert_b", [H], F32, kind="ExternalInput")
    d_sw = nc.dram_tensor("sparse_w", [H], BF16, kind="ExternalInput")
    d_sb = nc.dram_tensor("sparse_b", [1, 1], F32, kind="ExternalInput")
    d_oc = nc.dram_tensor("ones_col", [128, 1], F32R, kind="ExternalInput")
    d_op = nc.dram_tensor("ones_pair", [128, 2, 1], F8, kind="ExternalInput")
    d_or = nc.dram_tensor("ones_row", [1, 128], F32R, kind="ExternalInput")
    d_lm = nc.dram_tensor("lmask", [128, 128], F32, kind="ExternalInput")

    o_dense = nc.dram_tensor("dense", [BQ, BP], F32, kind="ExternalOutput")
    o_sparse = nc.dram_tensor("sparse", [BQ, BP], F32, kind="ExternalOutput")
    o_colbert = nc.dram_tensor("colbert", [BQ, BP], F32, kind="ExternalOutput")

    with _TC(nc) as tc:
        for _ in range(repeats):
            _emit(nc, tc, d_qhT, d_phT, d_q8, d_p8, d_w8, d_qm, d_pm, d_qi,
                  d_pi, d_cb, d_sw, d_sb, d_oc, d_op, d_or, d_lm, o_dense,
                  o_sparse, o_colbert)
    return nc


def _emit(nc, tc, d_qhT, d_phT, d_q8, d_p8, d_w8, d_qm, d_pm, d_qi, d_pi,
          d_cb, d_sw, d_sb, d_oc, d_op, d_or, d_lm, o_dense, o_sparse,
          o_colbert):
    from contextlib import ExitStack
    es = ExitStack()
    with es:
        es.enter_context(nc.allow_low_precision(reason="fp22/f32r is the target precision"))
        # ---- pools -------------------------------------------------------
        persist = es.enter_context(tc.tile_pool(name="persist", bufs=1))
        wt_pool = es.enter_context(tc.tile_pool(name="wt", bufs=1))
        qcolT_pool = es.enter_context(tc.tile_pool(name="qcolT", bufs=1))
        dram = es.enter_context(tc.tile_pool(name="dram", bufs=1, space="DRAM"))
        dram2 = es.enter_context(tc.tile_pool(name="dram2", bufs=2, space="DRAM"))
        ps_mm = es.enter_context(tc.tile_pool(name="ps_mm", bufs=3, space="PSUM"))
        ps_ss = es.enter_context(tc.tile_pool(name="ps_ss", bufs=2, space="PSUM"))
        ps_misc = es.enter_context(tc.tile_pool(name="ps_misc", bufs=1, space="PSUM"))

        # ---- persistent small tiles --------------------------------------
        ones_c = persist.tile([128, 1], F32R, tag="ones_c")
        nc.sync.dma_start(out=ones_c[:], in_=d_oc[:])
        ones_p = persist.tile([128, 2, 1], F8, tag="ones_p")
        nc.sync.dma_start(out=ones_p[:], in_=d_op[:])
        ones_r = persist.tile([1, 128], F32R, tag="ones_r")
        nc.sync.dma_start(out=ones_r[:], in_=d_or[:])
        lmask = persist.tile([128, 128], F32, tag="lmask")
        nc.sync.dma_start(out=lmask[:], in_=d_lm[:])
        sb_sb = persist.tile([1, 1], F32, tag="sb")
        nc.sync.dma_start(out=sb_sb[:], in_=d_sb[:])
        # colbert bias chunks: cb_sb[:, m] = b[m*128:(m+1)*128]
        cb_sb = persist.tile([128, HC], F32, tag="cb")
        nc.sync.dma_start(out=cb_sb[:], in_=d_cb.ap().rearrange("(m p) -> p m", p=128))
        # sparse_w chunks as lhsT columns: sw_sb[:, k] = sw[k*128:(k+1)*128]
        sw_sb = persist.tile([128, HC], BF16, tag="sw")
        nc.sync.dma_start(out=sw_sb[:], in_=d_sw.ap().rearrange("(k p) -> p k", p=128))
        # q ids as per-token columns: qid_cols[:, i] = q_ids_f[i, :]
        qid_cols = persist.tile([128, BQ], F32, tag="qid_cols")
        nc.sync.dma_start(out=qid_cols[:], in_=d_qi.ap().rearrange("i l -> l i"))
        # q mask tokens 1..127 transposed [token-1, batch] for qlen
        qmT = persist.tile([128, BQ], F32R, tag="qmT")
        nc.sync.dma_start(
            out=qmT[0:127, :],
            in_=bass.AP(tensor=d_qm.ap().tensor, offset=1,
                        ap=[[1, 127], [128, BQ]]))

        qcls_t = persist.tile([128, HC, BQ], BF16, tag="qcls")
        qcls = [qcls_t[:, k, :] for k in range(HC)]
        pcls_t = persist.tile([128, HC, BP], BF16, tag="pcls")
        pcls = [pcls_t[:, k, :] for k in range(HC)]
        rmax = [persist.tile([128, BP], F32R, tag=f"rmax{i}", name=f"rmax{i}")
                for i in range(BQ)]
        smax = [persist.tile([128, BP], F32R, tag=f"smax{i}", name=f"smax{i}")
                for i in range(BQ)]
        qw = [persist.tile([128, 1], F32R, tag=f"qw{i}", name=f"qw{i}")
              for i in range(BQ)]
        # per-(token,query) colbert final scale columns
        qnr = persist.tile([128, BQ], F32R, tag="qnr")
        twq_cols = persist.tile([128, BQ], F32R, tag="twq_cols")

        # fp8 pair-layout W (x W_SCALE): w8[g][:, t, m] = W[m, (2g+t)*128+p]
        w8_t = wt_pool.tile([128, HC // 2, 2, H], F8, tag="w8")
        w8 = [w8_t[:, g, :, :] for g in range(HC // 2)]
        # fp8 pair-layout q colbert vectors: qcolT8[g][:, t, :] holds hidden
        # chunk m=2g+t, scaled by FP8_SCALE (DoubleRow contracts 2x128=256)
        qcolT8 = [qcolT_pool.tile([128, 2, BQ * LQ], F8, tag=f"qct{g}",
                                  name=f"qct{g}") for g in range(HC // 2)]

        d_twq = dram.tile([1, BQ * LQ], F32R, name="d_twq")
        d_qnr = dram.tile([1, BQ * LQ], F32R, name="d_qnr")
        d_rq = dram.tile([1, BQ], F32, name="d_rq")

        # ================= SETUP: W8 (host-pre-transposed fp8 pairs) ======
        nc.sync.dma_start(out=w8_t[:], in_=d_w8.ap())

        # ================= SETUP: q side ==================================
        with tc.tile_pool(name="qhT", bufs=1) as qhT_pool, \
             tc.tile_pool(name="qtmp", bufs=1) as qtmp_pool, \
             tc.tile_pool(name="qv", bufs=2) as qv_pool:
            qhT_t = qhT_pool.tile([128, HC, BQ * 128], BF16, tag="qhT")
            nc.sync.dma_start(out=qhT_t[:], in_=d_qhT.ap())
            qhT = [qhT_t[:, k, :] for k in range(HC)]
            q8_t = qhT_pool.tile([128, HC // 2, 2, BQ * LQ], F8, tag="q8")
            nc.scalar.dma_start(out=q8_t[:], in_=d_q8.ap())
            q8 = [q8_t[:, g, :, :] for g in range(HC // 2)]
            # CLS columns (token 0 of each batch), all chunks in one copy
            nc.scalar.copy(
                out=qcls_t[:],
                in_=qhT_t.rearrange("p k (i l) -> p k i l", i=BQ)[:, :, :, 0])

            # token weights tw_q = relu(qh . sw + b), all 128 tokens per batch
            twq_row = qtmp_pool.tile([1, BQ * 128], F32R, tag="twq")
            for g in range(2):
                ptw = ps_ss.tile([1, 512], F32, tag="ss")
                for k in range(HC):
                    nc.tensor.matmul(ptw[:], sw_sb[:, k:k + 1],
                                     qhT[k][:, g * 512:(g + 1) * 512],
                                     start=(k == 0), stop=(k == HC - 1))
                nc.scalar.activation(out=twq_row[:, g * 512:(g + 1) * 512],
                                     in_=ptw[:], func=AF.Relu, bias=sb_sb[:], scale=1.0)
            # column form via DRAM bounce: twq_cols[l, i] = tw_q[i, l]
            nc.sync.dma_start(out=d_twq[:], in_=twq_row[:])
            nc.sync.dma_start(
                out=twq_cols[:],
                in_=bass.AP(tensor=d_twq.tensor, offset=0, ap=[[1, 128], [128, BQ]]))

            # ---- q dedup: keep, per (batch, id), the max-tw token --------
            for i in range(BQ):
                pb = ps_misc.tile([128, 128], F32, tag="misc")
                nc.tensor.matmul(pb[:], ones_r[:],
                                 twq_row[:, i * 128:(i + 1) * 128],
                                 start=True, stop=True)
                twB = qtmp_pool.tile([128, 128], F32, tag="twB")
                nc.scalar.copy(out=twB[:], in_=pb[:])
                tw_col = twq_cols[:, i:i + 1]
                qidB = qtmp_pool.tile([128, 128], F32, tag="qidB")
                nc.gpsimd.dma_start(out=qidB[:],
                                    in_=_bcast_rows(d_qi[i:i + 1, :]))
                # dominated(a) = sum_a' eq_id*(gt_tw + eq_tw*lower)
                x2 = qtmp_pool.tile([128, 128], F32, tag="x2")
                nc.vector.scalar_tensor_tensor(
                    out=x2[:], in0=twB[:], scalar=tw_col, in1=lmask[:],
                    op0=AX.is_equal, op1=AX.mult)
                x3 = qtmp_pool.tile([128, 128], F32, tag="x3")
                nc.vector.scalar_tensor_tensor(
                    out=x3[:], in0=twB[:], scalar=tw_col, in1=x2[:],
                    op0=AX.is_gt, op1=AX.add)
                dom = qtmp_pool.tile([128, 128], F32, tag="dom")
                dsum = qtmp_pool.tile([128, 1], F32, tag="dsum")
                nc.vector.scalar_tensor_tensor(
                    out=dom[:], in0=qidB[:], scalar=qid_cols[:, i:i + 1], in1=x3[:],
                    op0=AX.is_equal, op1=AX.mult, accum_out=dsum[:])
                surv = qtmp_pool.tile([128, 1], F32, tag="surv")
                nc.vector.tensor_scalar(out=surv[:], in0=dsum[:], scalar1=0.0,
                                        scalar2=None, op0=AX.is_equal)
                nc.vector.tensor_mul(qw[i][:], surv[:], tw_col)

            # ---- q colbert projection: raw v cast straight to fp8 pairs;
            # per-token 1/norm goes into the qnr finals column instead.
            qn_row = qtmp_pool.tile([1, BQ * 128], F32, tag="qn_row")
            for g in range(2):
                vsq8 = [qv_pool.tile([128, 2, 512], F8, tag=f"qvs{gg}",
                                     name=f"qvs{gg}") for gg in range(HC // 2)]
                pss = ps_ss.tile([1, 512], F32, tag="ss")
                for m in range(HC):
                    pmm = ps_mm.tile([128, 512], F32, tag="mm")
                    for kg in range(HC // 2):
                        nc.tensor.matmul(pmm[:],
                                         w8[kg][:, :, m * 128:(m + 1) * 128],
                                         q8[kg][:, :, g * 512:(g + 1) * 512],
                                         start=(kg == 0), stop=(kg == HC // 2 - 1),
                                         perf_mode=PM.DoubleRow)
                    nc.scalar.activation(
                        out=qcolT8[m // 2][:, m % 2, g * 512:(g + 1) * 512],
                        in_=pmm[:], func=AF.Identity,
                        bias=cb_sb[:, m:m + 1], scale=1.0 / W_SCALE)
                    nc.scalar.activation(out=vsq8[m // 2][:, m % 2, :], in_=pmm[:],
                                         func=AF.Square,
                                         bias=cb_sb[:, m:m + 1], scale=1.0 / W_SCALE)
                for gg in range(HC // 2):
                    for t in range(2):
                        nc.tensor.matmul(pss[:], ones_p[:, 0, :],
                                         vsq8[gg][:, t, :],
                                         start=(gg == 0 and t == 0),
                                         stop=(gg == HC // 2 - 1 and t == 1))
                nc.scalar.activation(out=qn_row[:, g * 512:(g + 1) * 512],
                                     in_=pss[:], func=AF.Sqrt)
            # qnr0 = qmask / ||v|| as a [1, BQ*LQ] row
            rq_full = qtmp_pool.tile([1, BQ * 128], F32R, tag="rq_full")
            nc.vector.reciprocal(out=rq_full[:], in_=qn_row[:])
            mrow = qtmp_pool.tile([1, BQ * 128], F32R, tag="mrow")
            nc.sync.dma_start(
                out=mrow[:],
                in_=bass.AP(tensor=d_qm.ap().tensor, offset=0,
                            ap=[[0, 1], [1, BQ * 128]]))
            nc.vector.tensor_mul(rq_full[:], rq_full[:], mrow[:])

            # qnr[a-1, i] = qmask[i,a] / (||v_q[i,a]|| * qlen_i * TEMP)
            # (tokens 1..127 on partitions 0..126, via DRAM bounce)
            nc.sync.dma_start(out=d_qnr[:], in_=rq_full[:])
            nc.sync.dma_start(
                out=qnr[0:127, :],
                in_=bass.AP(tensor=d_qnr.tensor, offset=1,
                            ap=[[1, 127], [128, BQ]]))
            pql = ps_ss.tile([1, BQ], F32, tag="ss")
            nc.tensor.matmul(pql[:], ones_c[0:127, :], qmT[0:127, :],
                             start=True, stop=True)
            qiv_row = qtmp_pool.tile([1, BQ], F32R, tag="qiv")
            nc.vector.tensor_scalar(out=qiv_row[:], in0=pql[:],
                                    scalar1=TEMP * FP8_SCALE,
                                    scalar2=None, op0=AX.mult)
            nc.vector.reciprocal(out=qiv_row[:], in_=qiv_row[:])
            pqb = ps_misc.tile([127, BQ], F32, tag="misc")
            nc.tensor.matmul(pqb[:], ones_r[:, 0:127], qiv_row[:],
                             start=True, stop=True)
            qivB = qtmp_pool.tile([128, BQ], F32, tag="qivB")
            nc.scalar.copy(out=qivB[0:127, :], in_=pqb[:])
            nc.vector.tensor_mul(qnr[0:127, :], qnr[0:127, :], qivB[0:127, :])

        # ================= MAIN LOOP over passages ========================
        # Software-pipelined: stage A(j) = DMA+transpose+project+normalize
        # (produces pcolT_j, twpB_j, pidB_j), stage B(j) = colbert scores +
        # sparse match. Emission order A0, A1, B0, A2, B1, ... keeps the PE
        # stream free of stalls: B(j)'s operands are ready by the time the
        # in-order PE queue reaches them.
        with tc.tile_pool(name="phT", bufs=2) as phT_pool, \
             tc.tile_pool(name="pcs", bufs=2) as pcs_pool, \
             tc.tile_pool(name="pvq", bufs=2) as pvq_pool, \
             tc.tile_pool(name="pcolT", bufs=2) as pcolT_pool, \
             tc.tile_pool(name="prow2", bufs=2) as prow2_pool, \
             tc.tile_pool(name="prow", bufs=1) as prow_pool:

            def stage_a(j):
                phT_t = phT_pool.tile([128, HC, LP], BF16, tag="phT")
                nc.sync.dma_start(out=phT_t[:], in_=d_phT[j])
                phT = [phT_t[:, k, :] for k in range(HC)]
                p8 = phT_pool.tile([128, HC // 2, 2, LP], F8, tag="p8")
                nc.sync.dma_start(out=p8[:], in_=d_p8[j])
                nc.scalar.copy(out=pcls_t[:, :, j], in_=phT_t[:, :, 0])

                # token weights tw_p = relu(ph . sw + b)
                ptw = ps_ss.tile([1, LP], F32, tag="ss")
                for k in range(HC):
                    nc.tensor.matmul(ptw[:], sw_sb[:, k:k + 1], phT[k][:],
                                     start=(k == 0), stop=(k == HC - 1))
                twp_row = prow_pool.tile([1, LP], F32R, tag="twp")
                nc.scalar.activation(out=twp_row[:], in_=ptw[:], func=AF.Relu,
                                     bias=sb_sb[:], scale=1.0)
                d_twb = dram2.tile([1, LP], F32R, tag="d_twb")
                nc.sync.dma_start(out=d_twb[:], in_=twp_row[:])
                twpB = prow2_pool.tile([128, LP], F32R, tag="twpB")
                nc.sync.dma_start(out=twpB[:], in_=_bcast_rows(d_twb[:]))
                pidB = prow2_pool.tile([128, LP], F32, tag="pidB")
                nc.gpsimd.dma_start(out=pidB[:], in_=_bcast_rows(d_pi[j:j + 1, :]))

                # colbert projection (all 512 tokens): raw v -> fp8 pairs
                pcolT8 = [pcolT_pool.tile([128, 2, LP], F8, tag=f"pct{g}",
                                          name=f"pct{g}_{j}")
                          for g in range(HC // 2)]
                vsq8 = [pvq_pool.tile([128, 2, LP], F8, tag=f"pvs{gg}",
                                      name=f"pvs{gg}") for gg in range(HC // 2)]
                pss = ps_ss.tile([1, LP], F32, tag="ss")
                for m in range(HC):
                    pmm = ps_mm.tile([128, LP], F32, tag="mm")
                    for kg in range(HC // 2):
                        nc.tensor.matmul(pmm[:],
                                         w8[kg][:, :, m * 128:(m + 1) * 128],
                                         p8[:, kg, :, :],
                                         start=(kg == 0), stop=(kg == HC // 2 - 1),
                                         perf_mode=PM.DoubleRow)
                    nc.scalar.activation(out=pcolT8[m // 2][:, m % 2, :],
                                         in_=pmm[:], func=AF.Identity,
                                         bias=cb_sb[:, m:m + 1], scale=1.0 / W_SCALE)
                    nc.scalar.activation(out=vsq8[m // 2][:, m % 2, :], in_=pmm[:],
                                         func=AF.Square,
                                         bias=cb_sb[:, m:m + 1], scale=1.0 / W_SCALE)
                for gg in range(HC // 2):
                    for t in range(2):
                        nc.tensor.matmul(pss[:], ones_p[:, 0, :],
                                         vsq8[gg][:, t, :],
                                         start=(gg == 0 and t == 0),
                                         stop=(gg == HC // 2 - 1 and t == 1))
                nrow = prow_pool.tile([1, LP], F32, tag="nrow")
                nc.scalar.activation(out=nrow[:], in_=pss[:], func=AF.Sqrt,
                                     scale=1.0 / (FP8_SCALE * FP8_SCALE))
                rrow = prow_pool.tile([1, LP], F32, tag="rrow")
                nc.vector.reciprocal(out=rrow[:], in_=nrow[:])
                rp_row = prow_pool.tile([1, LP], F32R, tag="rp_row")
                mrow = prow_pool.tile([1, LP], F32R, tag="mrow")
                nc.scalar.dma_start(out=mrow[:], in_=d_pm[j:j + 1, :])
                nc.vector.tensor_mul(rp_row[:], rrow[:], mrow[:])
                d_rpb = dram2.tile([1, LP], F32R, tag="d_rpb")
                nc.sync.dma_start(out=d_rpb[:], in_=rp_row[:])
                rpB = prow2_pool.tile([128, LP], F32R, tag="rpB")
                nc.sync.dma_start(out=rpB[:], in_=_bcast_rows(d_rpb[:]))
                # normalize on Pool: pcolT8s = fp8(v8 * FP8_SCALE*mask/||v||)
                pcolT8s = [pcs_pool.tile([128, 2, LP], F8, tag=f"pcs{g}",
                                         name=f"pcs{g}_{j}")
                           for g in range(HC // 2)]
                for g in range(HC // 2):
                    for t in range(2):
                        nc.vector.tensor_mul(pcolT8s[g][:, t, :],
                                             pcolT8[g][:, t, :], rpB[:])
                return pcolT8s, twpB, pidB

            def stage_b(j, st):
                pcolT8s, twpB, pidB = st
                for i in range(BQ):
                    psc = ps_mm.tile([127, LP], F32, tag="mm")
                    for g in range(HC // 2):
                        nc.tensor.matmul(
                            psc[:],
                            qcolT8[g][:, :, i * 128 + 1:(i + 1) * 128],
                            pcolT8s[g][:, :, :],
                            start=(g == 0), stop=(g == HC // 2 - 1),
                            perf_mode=PM.DoubleRow)
                    nc.vector.reduce_max(out=rmax[i][0:127, j:j + 1],
                                         in_=psc[:, 1:LP], axis=X)
                    mt = prow_pool.tile([128, LP], F32, tag="mt")
                    nc.vector.scalar_tensor_tensor(
                        out=mt[:], in0=pidB[:], scalar=qid_cols[:, i:i + 1],
                        in1=twpB[:], op0=AX.is_equal, op1=AX.mult)
                    nc.vector.reduce_max(out=smax[i][:, j:j + 1], in_=mt[:], axis=X)

            pending = stage_a(0)
            for j in range(1, BP):
                nxt = stage_a(j)
                stage_b(j - 1, pending)
                pending = nxt
            stage_b(BP - 1, pending)

        # ================= FINALS =========================================
        with tc.tile_pool(name="fin", bufs=1) as fin:
            for i in range(BQ):
                pcbi = ps_ss.tile([1, BP], F32, tag="ss")
                nc.tensor.matmul(pcbi[:], qnr[0:127, i:i + 1],
                                 rmax[i][0:127, :], start=True, stop=True)
                stag = fin.tile([1, BP], F32, tag=f"cst{i}", name=f"cst{i}")
                nc.scalar.copy(out=stag[:], in_=pcbi[:])
                nc.sync.dma_start(out=o_colbert[i:i + 1, :], in_=stag[:])

                pspi = ps_ss.tile([1, BP], F32, tag="ss")
                nc.tensor.matmul(pspi[:], qw[i][:], smax[i][:],
                                 start=True, stop=True)
                stag2 = fin.tile([1, BP], F32, tag=f"sst{i}", name=f"sst{i}")
                nc.scalar.activation(out=stag2[:], in_=pspi[:], func=AF.Copy,
                                     scale=1.0 / TEMP)
                nc.sync.dma_start(out=o_sparse[i:i + 1, :], in_=stag2[:])

            # dense scores
            pd = ps_misc.tile([BQ, BP], F32, tag="misc")
            pqn = ps_ss.tile([1, BQ], F32, tag="ss")
            ppn = ps_ss.tile([1, BP], F32, tag="ss")
            for k in range(HC):
                nc.tensor.matmul(pd[:], qcls[k][:], pcls[k][:],
                                 start=(k == 0), stop=(k == HC - 1))
                qsq = fin.tile([128, BQ], F32R, tag="qsq")
                nc.scalar.activation(out=qsq[:], in_=qcls[k][:], func=AF.Square)
                nc.tensor.matmul(pqn[:], ones_c[:], qsq[:],
                                 start=(k == 0), stop=(k == HC - 1))
                psq = fin.tile([128, BP], F32R, tag="psq")
                nc.scalar.activation(out=psq[:], in_=pcls[k][:], func=AF.Square)
                nc.tensor.matmul(ppn[:], ones_c[:], psq[:],
                                 start=(k == 0), stop=(k == HC - 1))
            pdsb = fin.tile([BQ, BP], F32, tag="pdsb")
            nc.scalar.copy(out=pdsb[:], in_=pd[:])
            rq_row = fin.tile([1, BQ], F32, tag="rq_row")
            nc.scalar.activation(out=rq_row[:], in_=pqn[:], func=AF.Sqrt)
            nc.vector.tensor_scalar(out=rq_row[:], in0=rq_row[:], scalar1=1e-12,
                                    scalar2=None, op0=AX.max)
            nc.vector.reciprocal(out=rq_row[:], in_=rq_row[:])
            rp_row = fin.tile([1, BP], F32R, tag="rp_row")
            nc.scalar.activation(out=rp_row[:], in_=ppn[:], func=AF.Sqrt)
            nc.vector.tensor_scalar(out=rp_row[:], in0=rp_row[:], scalar1=1e-12,
                                    scalar2=None, op0=AX.max)
            nc.vector.reciprocal(out=rp_row[:], in_=rp_row[:])
            # rq as a column via DRAM bounce
            nc.sync.dma_start(out=d_rq[:], in_=rq_row[:])
            rq_col = fin.tile([BQ, 1], F32, tag="rq_col")
            nc.sync.dma_start(
                out=rq_col[:],
                in_=bass.AP(tensor=d_rq.tensor, offset=0, ap=[[1, BQ], [0, 1]]))
            # rp broadcast across 8 partitions
            prpb = ps_misc.tile([BQ, BP], F32, tag="misc")
            nc.tensor.matmul(prpb[:], ones_r[:, 0:BQ], rp_row[:],
                             start=True, stop=True)
            rpB = fin.tile([BQ, BP], F32, tag="rpB")
            nc.scalar.copy(out=rpB[:], in_=prpb[:])
            dmul = fin.tile([BQ, BP], F32, tag="dmul")
            nc.vector.tensor_mul(dmul[:], pdsb[:], rpB[:])
            dout = fin.tile([BQ, BP], F32, tag="dout")
            nc.vector.tensor_scalar(out=dout[:], in0=dmul[:], scalar1=rq_col[:],
                                    scalar2=1.0 / TEMP, op0=AX.mult, op1=AX.mult)
            nc.sync.dma_start(out=o_dense[:], in_=dout[:])


# ---------------------------------------------------------------------------
# Host-side driver
# ---------------------------------------------------------------------------
_PROGRAM = None


def _get_program():
    global _PROGRAM
    if _PROGRAM is None:
        _PROGRAM = _build_program()
    return _PROGRAM


def _prep_ids(ids, sentinel):
    f = ids.astype(np.float32)
    return np.where(ids <= 3, np.float32(sentinel), f).astype(np.float32)


def make_in_maps(q_hidden, p_hidden, q_mask, p_mask, q_ids, p_ids,
                 colbert_w, colbert_b, sparse_w, sparse_b):
    import ml_dtypes
    q_hidden = np.asarray(q_hidden, np.float32)
    p_hidden = np.asarray(p_hidden, np.float32)
    q_mask = np.ascontiguousarray(np.asarray(q_mask, np.float32))
    p_mask = np.ascontiguousarray(np.asarray(p_mask, np.float32))
    colbert_w = np.asarray(colbert_w, np.float32)
    colbert_b = np.ascontiguousarray(np.asarray(colbert_b, np.float32))
    sparse_w = np.ascontiguousarray(np.asarray(sparse_w, np.float32))
    sparse_b = np.asarray(sparse_b, np.float32).reshape(1, 1)
    q_ids = np.asarray(q_ids)
    p_ids = np.asarray(p_ids)
    qi = _prep_ids(q_ids, -2.0)
    ones_col = np.ones((128, 1), np.float32)
    ones_pair = np.ones((128, 2, 1), ml_dtypes.float8_e4m3)
    ones_row = np.ones((1, 128), np.float32)
    a = np.arange(128)
    lmask = (a[None, :] < a[:, None]).astype(np.float32)  # [a, a'] = a' < a

    bf16 = ml_dtypes.bfloat16
    f8 = ml_dtypes.float8_e4m3

    # Host-side layout transforms (pure data movement + dtype casts),
    # partition-major so each DMA is 128 contiguous descriptors:
    # q_hiddenT[p, k, i*LQ+l] = q_hidden[i, l, k*128+p]
    qhT = np.ascontiguousarray(
        q_hidden.transpose(2, 0, 1).reshape(HC, 128, BQ * LQ)
        .transpose(1, 0, 2).astype(bf16))
    # p_hiddenT[j, p, k, l] = p_hidden[j, l, k*128+p]
    phT = np.ascontiguousarray(
        p_hidden.transpose(0, 2, 1).reshape(BP_FULL, HC, 128, LP)
        .transpose(0, 2, 1, 3).astype(bf16))
    # fp8 DoubleRow pair layouts: hidden index h = (2g+t)*128+p
    # q_hidden8[p, g, t, i*LQ+l]
    q8 = np.ascontiguousarray(
        q_hidden.transpose(2, 0, 1).reshape(HC // 2, 2, 128, BQ * LQ)
        .transpose(2, 0, 1, 3).astype(f8))
    # p_hidden8[j, p, g, t, l]
    p8 = np.ascontiguousarray(
        p_hidden.transpose(0, 2, 1).reshape(BP_FULL, HC // 2, 2, 128, LP)
        .transpose(0, 3, 1, 2, 4).astype(f8))
    # colbert_w8[p, g, t, m] = W_SCALE * colbert_w[m, (2g+t)*128+p]
    w8 = np.ascontiguousarray(
        (colbert_w.T * W_SCALE).reshape(HC // 2, 2, 128, H)
        .transpose(2, 0, 1, 3).astype(f8))

    in_maps = []
    for c in range(N_CORES):
        sl = slice(c * BP, (c + 1) * BP)
        in_maps.append({
            "q_hiddenT": qhT,
            "p_hiddenT": phT[sl],
            "q_hidden8": q8,
            "p_hidden8": p8[sl],
            "colbert_w8": w8,
            "q_mask": q_mask,
            "p_mask": np.ascontiguousarray(p_mask[sl]),
            "q_ids_f": qi,
            "p_ids_f": np.ascontiguousarray(_prep_ids(p_ids[sl], -1.0)),
            "colbert_b": colbert_b,
            "sparse_w": sparse_w.astype(bf16),
            "sparse_b": sparse_b,
            "ones_col": ones_col,
            "ones_pair": ones_pair,
            "ones_row": ones_row,
            "lmask": lmask,
        })
    return in_maps


def kernel(q_hidden, p_hidden, q_mask, p_mask, q_ids, p_ids,
           colbert_w, colbert_b, sparse_w, sparse_b):
    nc = _get_program()
    in_maps = make_in_maps(q_hidden, p_hidden, q_mask, p_mask, q_ids, p_ids,
                           colbert_w, colbert_b, sparse_w, sparse_b)
    res = run_bass_kernel_spmd(nc, in_maps, list(range(N_CORES)))
    dense = np.concatenate([res.results[c]["dense"] for c in range(N_CORES)], axis=1)
    sparse = np.concatenate([res.results[c]["sparse"] for c in range(N_CORES)], axis=1)
    colbert = np.concatenate([res.results[c]["colbert"] for c in range(N_CORES)],
                             axis=1)
    return dense, sparse, colbert

